# revision 51
# baseline (speedup 1.0000x reference)
"""Trainium2 Bass kernel for nn_Net_89361089561102 (2-layer dense transformer,
NF4-quantized weights, cls head). Tensor-parallel over 8 NeuronCores.

Strategy (v2):
 - Host: unpack NF4 weights -> bf16, pre-transpose to [K, M] layout, shard
   TP-style (qkv/gate_up by output dim == heads/ff, o by output dim, down
   INPUT-sharded by ff), embedding gather, RoPE cos/sin tables, causal masks.
 - Device layer 0 (full tokens), per-batch pipelined so collectives hide
   under the other batch's compute:
     ln1 (local) -> qkv + RoPE -> attention -> per-batch: [ctx AllGather,
     o_proj (output-sharded), residual, sumsq AllReduce, x AllGather,
     ln2, gate/up (local intermediate kept in SBUF), down (INPUT-sharded,
     no intermediate AllGather), ReduceScatter, residual, sumsq AllReduce,
     x AllGather].
 - Layer 1 (last layer): k/v full (per-batch passes so they start as soon
   as that batch's x AllGather lands); q/attn/o/MLP only at the last token
   of each batch; final rmsnorm + cls head redundant on every core.
"""

import math
from contextlib import ExitStack
from dataclasses import dataclass

import numpy as np
import ml_dtypes

BF16 = ml_dtypes.bfloat16
FP8 = ml_dtypes.float8_e4m3
W8SCALE = 64.0
EPS = 1e-5
BLK = 64
NF4 = np.array([
    -1.0, -0.6961928009986877, -0.5250730514526367, -0.39491748809814453,
    -0.28444138169288635, -0.18477343022823334, -0.09105003625154495, 0.0,
    0.07958029955625534, 0.16093020141124725, 0.24611230194568634,
    0.33791524171829224, 0.44070982933044434, 0.5626170039176941,
    0.7229568362236023, 1.0], dtype=np.float32)


@dataclass(frozen=True)
class Cfg:
    H: int
    NH: int
    HD: int
    FF: int
    B: int
    S: int
    L: int
    NC: int
    CLS: int = 768
    NCLS: int = 2
    P: int = 128

    @property
    def T(self):
        return self.B * self.S

    @property
    def KT(self):
        return self.H // self.P

    @property
    def KTF(self):
        return self.FF // self.P

    @property
    def HPC(self):  # heads per core
        return self.NH // self.NC

    @property
    def DR(self):  # q/k/v rows per core
        return self.HPC * self.HD

    @property
    def OR(self):  # o/down output rows per core
        return self.H // self.NC

    @property
    def OT(self):
        return self.OR // self.P

    @property
    def FPC(self):  # ff rows per core
        return self.FF // self.NC

    @property
    def FT(self):
        return self.FPC // self.P

    @property
    def SP(self):  # seq tiles per batch
        return self.S // self.P

    @property
    def TP_(self):  # token tiles total
        return self.T // self.P

    @property
    def CT(self):
        return self.CLS // self.P

    def check(self):
        assert self.H % self.P == 0 and self.FF % self.P == 0
        assert self.S % self.P == 0 and self.S <= 512
        assert self.NH % self.NC == 0 and self.H % self.NC == 0
        assert self.FF % self.NC == 0
        assert self.OR % self.P == 0 and self.FPC % self.P == 0
        assert self.HD <= self.P and self.HD % 2 == 0
        assert self.HPC * self.B <= 8  # q/k psum groups
        assert self.SP <= 8            # v psum groups (per batch)
        assert self.OT <= 8            # o psum groups (per batch)
        assert self.FT <= 8            # gate/up psum groups (per batch)
        assert self.CLS % self.P == 0


FULL_CFG = Cfg(H=3072, NH=32, HD=96, FF=8192, B=2, S=512, L=2, NC=8)


# ----------------------------------------------------------------------------
# host-side prep
# ----------------------------------------------------------------------------

def dequant_np(packed, absmax, out_f, in_f):
    shifts = (np.arange(8, dtype=np.int32) * 4)
    codes = ((packed[:, None] >> shifts) & 0xF).reshape(-1)
    w = (NF4[codes].reshape(-1, BLK) * absmax[:, None].astype(np.float32))
    return w.reshape(out_f, in_f)


def _wt3(w_t, P):
    """[K, M] fp32 -> [K//P, P, M] bf16 contiguous."""
    K, M = w_t.shape
    return np.ascontiguousarray(w_t.reshape(K // P, P, M).astype(BF16))


def host_prep(cfg: Cfg, inputs):
    """Full inputs -> list of per-core input maps."""
    c = cfg
    P = c.P
    x = inputs["embed"][inputs["input_ids"]]          # [B, S, H] fp32
    x0f = np.ascontiguousarray(x.reshape(c.T, c.H).T.astype(np.float32))
    x0 = np.ascontiguousarray(x0f.astype(BF16))        # [H, T] bf16

    # rope tables
    inv = 1.0 / (10000.0 ** (np.arange(0, c.HD, 2, dtype=np.float32) / c.HD))
    f = np.outer(np.arange(c.S, dtype=np.float32), inv)
    emb = np.concatenate([f, f], -1)                   # [S, HD]
    sgn = np.concatenate([-np.ones(c.HD // 2, np.float32),
                          np.ones(c.HD // 2, np.float32)])
    cosT = np.tile(np.cos(emb).T, (1, c.B))            # [HD, T]
    sinT = np.tile(np.sin(emb).T * sgn[:, None], (1, c.B))
    last = np.array([b * c.S + c.S - 1 for b in range(c.B)])
    cosT2 = np.ascontiguousarray(cosT[:, last].astype(np.float32))
    sinT2 = np.ascontiguousarray(sinT[:, last].astype(np.float32))
    cosT = np.ascontiguousarray(cosT.astype(BF16))
    sinT = np.ascontiguousarray(sinT.astype(BF16))

    am = (inputs["attention_mask"] != 0)               # [B, S]
    tk = np.arange(c.S)
    EW = c.S + (c.SP - 1) * P
    jj = np.arange(EW)
    m1 = (np.arange(P)[:, None] <= jj[None, :] - (c.SP - 1) * P)
    m1 = np.ascontiguousarray(m1.astype(BF16))
    am2 = np.zeros((c.B, P, c.SP), np.float32)
    for b in range(c.B):
        am2[b] = am[b].reshape(c.SP, P).T
    am2 = am2.astype(BF16)

    # layernorm weights [5, P, KT]
    lnw = np.zeros((2 * c.L + 1, P, c.KT), np.float32)
    for l in range(c.L):
        lnw[2 * l] = inputs["ln1_w"][l].reshape(c.KT, P).T
        lnw[2 * l + 1] = inputs["ln2_w"][l].reshape(c.KT, P).T
    lnw[2 * c.L] = inputs["final_ln_w"].reshape(c.KT, P).T

    # cls head: fold ln_g into w2, ln_b@w2.T+b2 into the output bias
    w1t = _wt3(inputs["w1"].astype(np.float32).T
               * inputs["final_ln_w"].astype(np.float32)[:, None],
               P)                                          # [KT, P, CLS]
    b1row = np.ascontiguousarray(
        inputs["b1"].reshape(1, c.CLS).astype(BF16))
    w2_f = inputs["w2"].astype(np.float32)                 # [NCLS, CLS]
    w2g = np.ascontiguousarray(
        (w2_f * inputs["ln_g"][None, :]).T.reshape(c.CT, P, c.NCLS)
        .astype(BF16))
    b2row = np.ascontiguousarray(
        (inputs["ln_b"].astype(np.float32) @ w2_f.T
         + inputs["b2"].astype(np.float32)).reshape(1, c.NCLS))

    shared = dict(x0=x0, cosT=cosT, sinT=sinT, cosT2=cosT2, sinT2=sinT2,
                  m1=m1, am2=am2, lnw=lnw, w1t=w1t, b1row=b1row, w2g=w2g,
                  b2row=b2row, ident2=np.eye(2, dtype=np.float32))

    # per-layer dequantized weights (full) then shard
    per_layer = []
    for l in range(c.L):
        wqkv = dequant_np(inputs["qkv_packed"][l], inputs["qkv_absmax"][l],
                          3 * c.H, c.H)
        wo = dequant_np(inputs["o_packed"][l], inputs["o_absmax"][l],
                        c.H, c.H)
        wgu = dequant_np(inputs["gu_packed"][l], inputs["gu_absmax"][l],
                         2 * c.FF, c.H)
        wd = dequant_np(inputs["down_packed"][l], inputs["down_absmax"][l],
                        c.H, c.FF)
        per_layer.append((wqkv, wo, wgu, wd))

    in_maps = []
    for core in range(c.NC):
        m = dict(shared)
        m["x0r"] = np.ascontiguousarray(
            x0f[core * c.OR:(core + 1) * c.OR, :])
        for l in range(c.L):
            wqkv, wo, wgu, wd = per_layer[l]
            d0 = core * c.DR
            lf = inputs["ln1_w"][l].astype(np.float32)[None, :]

            def _pair8(w_t):
                K8, M8 = w_t.shape
                return np.ascontiguousarray(
                    (w_t * W8SCALE).reshape(K8 // (2 * P), 2, P, M8)
                    .transpose(0, 2, 1, 3).astype(FP8))
            m[f"wq{l}"] = _pair8((wqkv[d0:d0 + c.DR, :] * lf).T)
            m[f"wk{l}"] = _pair8((wqkv[c.H + d0:c.H + d0 + c.DR, :] * lf).T)
            m[f"wv{l}"] = _pair8(
                (wqkv[2 * c.H + d0:2 * c.H + d0 + c.DR, :] * lf).T)
            o0 = core * c.OR
            m[f"wo{l}"] = _pair8(wo[o0:o0 + c.OR, :].T)
            g0 = core * c.FPC

            def _wt8p(w_t):
                # [H, M] -> [H/2P, P, 2, M] fp8 (DoubleRow pair layout)
                K, M = w_t.shape
                arr = (w_t * W8SCALE).reshape(K // (2 * P), 2, P, M)
                return np.ascontiguousarray(
                    arr.transpose(0, 2, 1, 3).astype(FP8))
            if l == c.L - 1:
                m[f"wg{l}"] = _wt3(wgu[g0:g0 + c.FPC, :].T, P)
                m[f"wu{l}"] = _wt3(wgu[c.FF + g0:c.FF + g0 + c.FPC, :].T, P)
            else:
                m[f"wg{l}"] = _wt8p(wgu[g0:g0 + c.FPC, :].T)
                m[f"wu{l}"] = _wt8p(wgu[c.FF + g0:c.FF + g0 + c.FPC, :].T)
            if l == c.L - 1:
                # slim layer: INPUT-sharded down [FT, P, H] and o [DR/P, P, H]
                # stored fp8 (x W8SCALE) to halve the weight stream
                def _wt8(w_t):
                    K, M = w_t.shape
                    return np.ascontiguousarray(
                        (w_t * W8SCALE).reshape(K // P, P, M).astype(FP8))
                wdt = np.ascontiguousarray(wd[:, g0:g0 + c.FPC].T)
                m[f"wd{l}"] = _wt8(wdt)
                wot = np.ascontiguousarray(wo[:, d0:d0 + c.DR].T)
                m["wo2s"] = _wt8(wot)
                ln2c = inputs["ln2_w"][l].astype(np.float32).reshape(
                    c.KT, P, 1)
                m["wgu1"] = np.ascontiguousarray(np.concatenate(
                    [m[f"wg{l}"].astype(np.float32),
                     m[f"wu{l}"].astype(np.float32)], axis=2)
                    * ln2c * W8SCALE).astype(FP8)
                del m[f"wg{l}"], m[f"wu{l}"]
            else:
                # full layer: INPUT-sharded down, ot-major DoubleRow pairs
                # [OT=KT, P, FT/2, 2, P] fp8 (x W8SCALE)
                wdt = np.ascontiguousarray(wd[:, g0:g0 + c.FPC].T)  # [FPC, H]
                arr = (wdt * W8SCALE).reshape(c.FT // 2, 2, P, c.KT, P)
                arr = arr.transpose(3, 2, 0, 1, 4)
                m[f"wd{l}"] = np.ascontiguousarray(arr.astype(FP8))
        in_maps.append(m)
    return in_maps


# ----------------------------------------------------------------------------
# device kernel
# ----------------------------------------------------------------------------

def build_nc(cfg: Cfg):
    import concourse.bass as bass
    import concourse.mybir as mybir
    import concourse.tile as tile
    from concourse import bacc

    c = cfg
    c.check()
    P = c.P
    f32 = mybir.dt.float32
    bf16 = mybir.dt.bfloat16
    AF = mybir.ActivationFunctionType
    OP = mybir.AluOpType

    nc = bacc.Bacc("TRN2", target_bir_lowering=False, debug=False,
                   enable_asserts=False, num_devices=c.NC)
    RG = [list(range(c.NC))]
    SHARED = "Shared" if c.NC > 4 else "Local"

    def din(name, shape, dt):
        return nc.dram_tensor(name, list(shape), dt, kind="ExternalInput").ap()

    x0 = din("x0", [c.H, c.T], bf16)
    x0r = din("x0r", [c.OR, c.T], f32)
    cosT = din("cosT", [c.HD, c.T], bf16)
    sinT = din("sinT", [c.HD, c.T], bf16)
    cosT2 = din("cosT2", [c.HD, c.B], f32)
    sinT2 = din("sinT2", [c.HD, c.B], f32)
    m1 = din("m1", [P, c.S + (c.SP - 1) * P], bf16)
    am2 = din("am2", [c.B, P, c.SP], bf16)
    lnw_d = din("lnw", [2 * c.L + 1, P, c.KT], f32)
    w1t = din("w1t", [c.KT, P, c.CLS], bf16)
    b1r = din("b1row", [1, c.CLS], bf16)
    w2g = din("w2g", [c.CT, P, c.NCLS], bf16)
    b2r = din("b2row", [1, c.NCLS], f32)
    ident2 = din("ident2", [2, 2], f32)
    fp8 = mybir.dt.float8e4
    wq = [din(f"wq{l}", [c.KT // 2, P, 2, c.DR], fp8) for l in range(c.L)]
    wk = [din(f"wk{l}", [c.KT // 2, P, 2, c.DR], fp8) for l in range(c.L)]
    wv = [din(f"wv{l}", [c.KT // 2, P, 2, c.DR], fp8)
          for l in range(c.L)]
    wo = [din(f"wo{l}", [c.KT // 2, P, 2, c.OR], fp8)
          for l in range(c.L)]
    wg = [din(f"wg{l}", [c.KT // 2, P, 2, c.FPC], fp8)
          for l in range(c.L - 1)]
    wu = [din(f"wu{l}", [c.KT // 2, P, 2, c.FPC], fp8)
          for l in range(c.L - 1)]
    wgu1 = din("wgu1", [c.KT, P, 2 * c.FPC], fp8)
    wd = [din(f"wd{l}",
              [c.FT, P, c.H] if l == c.L - 1 else
              [c.KT, P, c.FT // 2, 2, P],
              fp8) for l in range(c.L)]
    wo2s = din("wo2s", [c.DR // P, P, c.H], fp8)
    out_d = nc.dram_tensor("logits_out", [c.NCLS, c.B], f32,
                           kind="ExternalOutput").ap()

    isqrt_hd = 1.0 / math.sqrt(c.HD)
    LAST = c.L - 1

    def lastcols(ap2d):
        """[P, T] AP -> [P, B] AP selecting the last token of each batch."""
        return ap2d.rearrange("p (b s) -> p b s", s=c.S)[:, :, c.S - 1]

    with tile.TileContext(nc) as tc, ExitStack() as ctx:
        const = ctx.enter_context(tc.tile_pool(name="const", bufs=1))
        persist = ctx.enter_context(tc.tile_pool(name="persist", bufs=1))
        wpool = ctx.enter_context(tc.tile_pool(name="wpool", bufs=3))
        xpool = ctx.enter_context(tc.tile_pool(name="xpool", bufs=3))
        spool = ctx.enter_context(tc.tile_pool(name="spool", bufs=2))
        ppool = ctx.enter_context(tc.tile_pool(name="ppool", bufs=3))
        rpool = ctx.enter_context(tc.tile_pool(name="rpool", bufs=1))
        mpool = ctx.enter_context(tc.tile_pool(name="mpool", bufs=2))
        psum = ctx.enter_context(tc.tile_pool(name="psum", bufs=8,
                                              space="PSUM"))
        dram = ctx.enter_context(tc.tile_pool(name="dram", bufs=1,
                                              space="DRAM"))

        # ---- constants in SBUF ----
        ones_c32 = const.tile([P, 1], f32, tag="ones_c32")
        nc.vector.memset(ones_c32[:], 1.0)
        ones_cbf = const.tile([P, 1], bf16, tag="ones_cbf")
        nc.vector.memset(ones_cbf[:], 1.0)
        ones_r32 = const.tile([1, P], f32, tag="ones_r32")
        nc.vector.memset(ones_r32[:], 1.0)
        eps_col = const.tile([P, 1], f32, tag="eps_col")
        nc.vector.memset(eps_col[:], EPS)
        cos_sb = const.tile([c.HD, c.T], bf16, tag="cos_sb")
        nc.scalar.dma_start(out=cos_sb[:], in_=cosT)
        sin_sb = const.tile([c.HD, c.T], bf16, tag="sin_sb")
        nc.scalar.dma_start(out=sin_sb[:], in_=sinT)
        cos2_sb = const.tile([c.HD, c.B], f32, tag="cos2_sb")
        nc.sync.dma_start(out=cos2_sb[:], in_=cosT2)
        sin2_sb = const.tile([c.HD, c.B], f32, tag="sin2_sb")
        nc.sync.dma_start(out=sin2_sb[:], in_=sinT2)
        mask_sb = const.tile([P, c.S + (c.SP - 1) * P], bf16,
                             tag="mask_sb")
        nc.scalar.dma_start(out=mask_sb[:], in_=m1)
        am2_sb = const.tile([P, c.B, c.SP], bf16, tag="am2_sb")
        for b in range(c.B):
            nc.sync.dma_start(out=am2_sb[:, b, :], in_=am2[b])
        lnw_sb = const.tile([P, 2 * c.L + 1, c.KT], f32, tag="lnw_sb")
        for n in range(2 * c.L + 1):
            nc.sync.dma_start(out=lnw_sb[:, n, :], in_=lnw_d[n])
        b1_sb = const.tile([1, c.CLS], bf16, tag="b1_sb")
        nc.sync.dma_start(out=b1_sb[:], in_=b1r)
        b2_sb = const.tile([1, c.NCLS], f32, tag="b2_sb")
        nc.sync.dma_start(out=b2_sb[:], in_=b2r)
        ones_rbf = const.tile([1, 2], bf16, tag="ones_rbf")
        nc.vector.memset(ones_rbf[:], 1.0)
        ones_rb = const.tile([1, P], bf16, tag="ones_rb")
        nc.vector.memset(ones_rb[:], 1.0)
        ones_r2 = const.tile([1, 2], f32, tag="ones_r2")
        nc.vector.memset(ones_r2[:], 1.0)
        id2 = const.tile([2, 2], f32, tag="id2")
        nc.sync.dma_start(out=id2[:], in_=ident2)
        id2b = const.tile([2, 2], bf16, tag="id2b")
        nc.vector.tensor_copy(id2b[:], id2[:])

        # ---- collective warm-up ----
        wu_sb = const.tile([P, 128], f32, tag="wu_sb")
        nc.vector.memset(wu_sb[:], 0.0)
        wu_in = dram.tile([P, 128], f32, tag="wu_in", name="wu_in")
        wu_out = dram.tile([P * c.NC, 128], f32, addr_space=SHARED,
                           tag="wu_out", name="wu_out")
        nc.sync.dma_start(out=wu_in[:], in_=wu_sb[:])
        nc.gpsimd.collective_compute(
            "AllGather", OP.bypass, replica_groups=RG,
            ins=[wu_in[:]], outs=[wu_out[:]])

        # ---- persistent activation state ----
        xn = persist.tile([P, c.KT, c.T], fp8, tag="xn")
        s12_col = const.tile([P, 1], f32, tag="s12_col")
        nc.vector.memset(s12_col[:], 1.0 / (W8SCALE * W8SCALE))
        s64_col = const.tile([P, 1], f32, tag="s64_col")
        nc.vector.memset(s64_col[:], 1.0 / W8SCALE)
        xrows = persist.tile([P, c.OT, c.T], f32, tag="xrows")
        for ot in range(c.OT):
            nc.scalar.dma_start(out=xrows[:, ot, :],
                                in_=x0r[ot * P:(ot + 1) * P, :])

        # ---------- helpers ----------
        def emit_norm(src_ap, lnidx, dst, ncols, chunks):
            """rmsnorm of src [H, ncols] (bf16) -> dst [P, KT, ncols]."""
            ss = [psum.tile([1, cw], f32, tag="ps", name=f"ssps{lnidx}_{ci}")
                  for ci, (c0, cw) in enumerate(chunks)]
            for kt in range(c.KT):
                xf = xpool.tile([P, ncols], bf16, tag="xf", name="xf", bufs=2)
                nc.sync.dma_start(out=xf[:], in_=src_ap[kt * P:(kt + 1) * P, :])
                nc.vector.tensor_copy(dst[:, kt, :], xf[:])
                sq = xpool.tile([P, ncols], bf16, tag="sq", name="sq", bufs=2)
                nc.vector.tensor_mul(sq[:], xf[:], xf[:])
                for ci, (c0, cw) in enumerate(chunks):
                    nc.tensor.matmul(ss[ci][:], ones_cbf[:], sq[:, c0:c0 + cw],
                                     start=(kt == 0), stop=(kt == c.KT - 1))
            bc = spool.tile([P, ncols], f32, tag="bc", name="bc", bufs=1)
            for ci, (c0, cw) in enumerate(chunks):
                lt = spool.tile([1, cw], f32, tag="lt", name="lt")
                nc.scalar.activation(lt[:], ss[ci][:], AF.Ln,
                                     bias=eps_col[0:1, :], scale=1.0 / c.H)
                rt = spool.tile([1, cw], f32, tag="rt", name="rt")
                nc.scalar.activation(rt[:], lt[:], AF.Exp, scale=-0.5)
                bb = psum.tile([P, cw], f32, tag="ps", name="bbps")
                nc.tensor.matmul(bb[:], ones_r32[:], rt[:],
                                 start=True, stop=True)
                nc.scalar.copy(bc[:, c0:c0 + cw], bb[:])
            for kt in range(c.KT):
                nc.vector.scalar_tensor_tensor(
                    dst[:, kt, :], dst[:, kt, :],
                    lnw_sb[:, lnidx, kt:kt + 1], bc[:],
                    OP.mult, OP.mult)

        def emit_rstd_bcast(ss_aps, ncols, chunks, pcount=None):
            """per-chunk [1, cw] full-H sumsq APs -> [P, ncols] rstd bcast."""
            pc = P if pcount is None else pcount
            bc = spool.tile([pc, ncols], f32, tag="bc", name="bc", bufs=1)
            for ci, (c0, cw) in enumerate(chunks):
                lt = spool.tile([1, cw], f32, tag="lt", name="lt")
                nc.scalar.activation(lt[:], ss_aps[ci], AF.Ln,
                                     bias=eps_col[0:1, :], scale=1.0 / c.H)
                rt = spool.tile([1, cw], f32, tag="rt", name="rt")
                nc.scalar.activation(rt[:], lt[:], AF.Exp, scale=-0.5)
                bb = psum.tile([pc, cw], f32, tag="ps", name="bbps")
                nc.tensor.matmul(bb[:], ones_r32[:, 0:pc], rt[:],
                                 start=True, stop=True)
                nc.scalar.copy(bc[:, c0:c0 + cw], bb[:])
            return bc

        def emit_sumsq_ar(cols0, ncols, tag):
            """Partial sum-of-squares of this core's fp32 x rows over
            [cols0, cols0+ncols), then a tiny AllReduce."""
            ssq = psum.tile([1, ncols], f32, tag="ps", name=f"ssA{tag}")
            for ot in range(c.OT):
                sqr = xpool.tile([P, ncols], bf16, tag="sq", name="sqr",
                                 bufs=2)
                nc.vector.tensor_mul(sqr[:], xrows[:, ot, cols0:cols0 + ncols],
                                     xrows[:, ot, cols0:cols0 + ncols])
                nc.tensor.matmul(ssq[:], ones_cbf[:], sqr[:],
                                 start=(ot == 0), stop=(ot == c.OT - 1))
            srow = spool.tile([1, ncols], f32, tag="srow", name="srow")
            nc.scalar.copy(srow[:], ssq[:])
            ssb = dram.tile([1, ncols], f32, tag=f"ssb{tag}",
                            name=f"ssb{tag}")
            ssg = dram.tile([1, ncols], f32, addr_space=SHARED,
                            tag=f"ssg{tag}", name=f"ssg{tag}")
            nc.sync.dma_start(out=ssb[:], in_=srow[:])
            nc.gpsimd.collective_compute(
                "AllReduce", OP.add, replica_groups=RG,
                ins=[ssb[:]], outs=[ssg[:]])
            return ssg

        def emit_norm_post(ssg, src_ap, lnidx, dst, cols0, ncols):
            """Normalize src [H, ncols] (bf16, DRAM) into
            dst[:, kt, cols0:cols0+ncols] using the AllReduced sumsq."""
            sst = spool.tile([1, ncols], f32, tag="sst", name="sst")
            nc.sync.dma_start(out=sst[:], in_=ssg[:])
            bc = emit_rstd_bcast([sst[:]], ncols, [(0, ncols)])
            for kt in range(c.KT):
                xf = xpool.tile([P, ncols], bf16, tag="xf", name="xfa",
                                bufs=2)
                nc.sync.dma_start(out=xf[:],
                                  in_=src_ap[kt * P:(kt + 1) * P, :])
                nc.vector.scalar_tensor_tensor(
                    dst[:, kt, cols0:cols0 + ncols], xf[:],
                    lnw_sb[:, lnidx, kt:kt + 1],
                    bc[:], OP.mult, OP.mult)

        def emit_norm_raw_cols(src_ap, dst, cols0, ncols):
            """Copy RAW src [H, ncols] (bf16 DRAM) into dst cols and
            return the rstd broadcast tile (norm folded downstream)."""
            ss = psum.tile([1, ncols], f32, tag="ps", name="ssc")
            for kt in range(c.KT):
                xf = xpool.tile([P, ncols], bf16, tag="xf", name="xfc",
                                bufs=2)
                nc.sync.dma_start(out=xf[:],
                                  in_=src_ap[kt * P:(kt + 1) * P, :])
                nc.vector.tensor_copy(dst[:, kt, cols0:cols0 + ncols],
                                      xf[:])
                sq = xpool.tile([P, ncols], bf16, tag="sq", name="sqc",
                                bufs=2)
                nc.vector.tensor_mul(sq[:], xf[:], xf[:])
                nc.tensor.matmul(ss[:], ones_cbf[:], sq[:],
                                 start=(kt == 0), stop=(kt == c.KT - 1))
            return emit_rstd_bcast([ss[:]], ncols, [(0, ncols)])

        def emit_norm_slim(src_ap, lnidx, dst3):
            """rmsnorm of a [H, B] tensor: one DMA + local sumsq."""
            xs = spool.tile([P, c.KT, c.B], bf16, tag="xs_slim",
                            name="xs_slim")
            nc.sync.dma_start(
                out=xs[:],
                in_=src_ap.rearrange("(kt p) b -> p kt b", p=P))
            sq = spool.tile([P, c.KT, c.B], f32, tag="sq_slim",
                            name="sq_slim")
            nc.vector.tensor_mul(sq[:], xs[:], xs[:])
            sp_ = psum.tile([1, c.KT * c.B], f32, tag="ps", name="spslim")
            nc.tensor.matmul(sp_[:], ones_c32[:],
                             sq[:].rearrange("p kt b -> p (kt b)"),
                             start=True, stop=True)
            ss2 = spool.tile([1, c.B], f32, tag="ss2", name="ss2")
            nc.vector.tensor_reduce(
                ss2[:], sp_[:].rearrange("o (kt b) -> o b kt", b=c.B),
                mybir.AxisListType.X, OP.add)
            bc = emit_rstd_bcast([ss2[:]], c.B, [(0, c.B)])
            tmp = spool.tile([P, c.KT, c.B], f32, tag="tmp_slim",
                             name="tmp_slim")
            nc.vector.tensor_tensor(
                tmp[:], xs[:],
                lnw_sb[:, lnidx, :].unsqueeze(2).broadcast_to(
                    (P, c.KT, c.B)), OP.mult)
            nc.vector.tensor_tensor(
                dst3[:], tmp[:],
                bc[:].unsqueeze(1).broadcast_to((P, c.KT, c.B)), OP.mult)

        def kouter_pass(KK, wsrc, wcols, groups, rhs_fn, rhs_load=None,
                        name="kp"):
            """Generic contraction pass: loop k tiles (batched weight DMA),
            stream weights, accumulate len(groups) psum tiles."""
            ps = [psum.tile([cw, n], f32, tag="ps", name=f"{name}{gi}")
                  for gi, (c0, cw, n, rk) in enumerate(groups)]
            G = max(1, min(4, 2048 // wcols))
            for k0 in range(0, KK, G):
                g_n = min(G, KK - k0)
                wt = wpool.tile([P, G, 2048 // G if wcols > 2048 // G else wcols],
                                bf16, tag="wt", name=f"{name}w")
                weng = nc.sync if (k0 // G) % 2 == 0 else nc.scalar
                weng.dma_start(
                    out=wt[:, 0:g_n, 0:wcols],
                    in_=wsrc(k0, g_n).rearrange("g p m -> p g m"))
                for g in range(g_n):
                    kt = k0 + g
                    rl = rhs_load(kt) if rhs_load is not None else None
                    for gi, (c0, cw, n, rk) in enumerate(groups):
                        nc.tensor.matmul(ps[gi][:], wt[:, g, c0:c0 + cw],
                                         rhs_fn(kt, rk, rl),
                                         start=(kt == 0), stop=(kt == KK - 1))
            return ps

        def qk_pair_pass(wsrc, groups, name):
            """DoubleRow q/k projection: out [cw, S] per group from
            fp8 pair weights and fp8 xn pairs."""
            ps = [psum.tile([cw, n], f32, tag="ps", name=f"{name}{gi}")
                  for gi, (c0, cw, n, rk) in enumerate(groups)]
            for ktp in range(c.KT // 2):
                wt8 = wpool.tile([P, 2, c.DR], fp8, tag="w8",
                                 name=f"{name}w", bufs=3)
                weng = nc.sync if ktp % 2 == 0 else nc.scalar
                weng.dma_start(out=wt8[:], in_=wsrc[ktp])
                for gi, (c0, cw, n, rk) in enumerate(groups):
                    nc.tensor.matmul(
                        ps[gi][:], wt8[:, :, c0:c0 + cw],
                        xn[:, 2 * ktp:2 * ktp + 2,
                           rk * c.S:(rk + 1) * c.S],
                        start=(ktp == 0), stop=(ktp == c.KT // 2 - 1),
                        perf_mode=mybir.MatmulPerfMode.DoubleRow)
            return ps

        def emit_rope(src_ps, qr_dst, cos_ap, sin_ap, ncols):
            """rope: qr_dst = src*cos + swap_half(src)*sin_signed."""
            h2 = c.HD // 2
            qs = rpool.tile([c.HD, ncols], f32, tag="qs", name="qs")
            nc.vector.tensor_copy(qs[:], src_ps[:])
            rot = rpool.tile([c.HD, ncols], f32, tag="rot", name="rot")
            nc.sync.dma_start(out=rot[0:h2, :], in_=qs[h2:c.HD, :])
            nc.sync.dma_start(out=rot[h2:c.HD, :], in_=qs[0:h2, :])
            nc.vector.tensor_mul(qs[:], qs[:], cos_ap)
            nc.vector.tensor_mul(rot[:], rot[:], sin_ap)
            nc.vector.tensor_add(qr_dst, qs[:], rot[:])

        # =====================================================
        # LAYER 0 (full tokens, per-batch pipelined)
        # =====================================================
        l = 0
        full_chunks = [(b * c.S, c.S) for b in range(c.B)]
        # ln1 folded: copy RAW x into xn, rstd applied via rope tables
        # and v-copy scale (ln1_w folded into wq/wk/wv host-side)
        ss1 = [psum.tile([1, cw], f32, tag="ps", name=f"ss1_{ci}")
               for ci, (c0, cw) in enumerate(full_chunks)]
        for kt in range(c.KT):
            xf = xpool.tile([P, c.T], bf16, tag="xf", name="xf", bufs=2)
            nc.sync.dma_start(out=xf[:], in_=x0[kt * P:(kt + 1) * P, :])
            nc.vector.tensor_copy(xn[:, kt, :], xf[:])
            sq = xpool.tile([P, c.T], bf16, tag="sq", name="sq", bufs=2)
            nc.vector.tensor_mul(sq[:], xf[:], xf[:])
            for ci, (c0, cw) in enumerate(full_chunks):
                nc.tensor.matmul(ss1[ci][:], ones_cbf[:], sq[:, c0:c0 + cw],
                                 start=(kt == 0), stop=(kt == c.KT - 1))
        bc1 = emit_rstd_bcast([s[:] for s in ss1], c.T, full_chunks)
        cosb = mpool.tile([c.HD, c.T], bf16, tag="cosb", name="cosb",
                          bufs=1)
        nc.vector.scalar_tensor_tensor(cosb[:], cos_sb[:],
                                       s64_col[0:c.HD, :],
                                       bc1[0:c.HD, :], OP.mult, OP.mult)
        sinb = mpool.tile([c.HD, c.T], bf16, tag="sinb", name="sinb",
                          bufs=1)
        nc.vector.scalar_tensor_tensor(sinb[:], sin_sb[:],
                                       s64_col[0:c.HD, :],
                                       bc1[0:c.HD, :], OP.mult, OP.mult)
        q_rot = persist.tile([c.HD, c.HPC, c.T], bf16, tag="qrot",
                             name=f"qrot{l}")
        k_rot = persist.tile([c.HD, c.HPC, c.T], bf16, tag="krot",
                             name=f"krot{l}")
        v_sb = persist.tile([P, c.TP_, c.HPC, c.HD + 1], bf16, tag="vsb",
                            name=f"vsb{l}")
        nc.vector.memset(v_sb[:, :, :, c.HD:c.HD + 1], 1.0)

        # q pass
        qg = [(h * c.HD, c.HD, c.S, b)
              for h in range(c.HPC) for b in range(c.B)]
        qrhs = lambda kt, rk, rl: xn[:, kt, rk * c.S:(rk + 1) * c.S]
        qps = qk_pair_pass(wq[l], qg, "qp")
        for gi, (c0, cw, n, rk) in enumerate(qg):
            h = c0 // c.HD
            emit_rope(qps[gi], q_rot[:, h, rk * c.S:(rk + 1) * c.S],
                      cosb[:, rk * c.S:(rk + 1) * c.S],
                      sinb[:, rk * c.S:(rk + 1) * c.S], c.S)

        # k pass
        kps = qk_pair_pass(wk[l], qg, "kp")
        # rstd as a token-partition column for the v-copy scale (issued
        # after q/k so the tiny transposes never stall the PE queue)
        rtp = psum.tile([P, c.TP_], f32, tag="ps", name="rtp")
        for tt in range(c.TP_):
            nc.tensor.matmul(rtp[:, tt:tt + 1],
                             bc1[0:1, tt * P:(tt + 1) * P],
                             id2[0:1, 0:1], start=True, stop=True,
                             is_transpose=True)
        rtc = spool.tile([P, c.TP_], f32, tag="rtc", name="rtc")
        nc.scalar.activation(rtc[:], rtp[:], AF.Copy,
                             scale=1.0 / W8SCALE)
        for gi, (c0, cw, n, rk) in enumerate(qg):
            h = c0 // c.HD
            emit_rope(kps[gi], k_rot[:, h, rk * c.S:(rk + 1) * c.S],
                      cosb[:, rk * c.S:(rk + 1) * c.S],
                      sinb[:, rk * c.S:(rk + 1) * c.S], c.S)

        # v pass (token-major)
        vps = [psum.tile([P, c.DR], f32, tag="ps", name=f"vp{tt}")
               for tt in range(c.TP_)]
        DRm = mybir.MatmulPerfMode.DoubleRow
        for ktp in range(c.KT // 2):
            wt = wpool.tile([P, 2, c.DR], fp8, tag="w8", name="vw")
            weng = nc.sync if ktp % 2 == 0 else nc.scalar
            weng.dma_start(out=wt[:], in_=wv[l][ktp])
            for tt in range(c.TP_):
                nc.tensor.matmul(vps[tt][:],
                                 xn[:, 2 * ktp:2 * ktp + 2,
                                    tt * P:(tt + 1) * P],
                                 wt[:],
                                 start=(ktp == 0),
                                 stop=(ktp == c.KT // 2 - 1),
                                 perf_mode=DRm)
        for tt in range(c.TP_):
            for h in range(c.HPC):
                nc.scalar.activation(v_sb[:, tt, h, 0:c.HD],
                                     vps[tt][:, h * c.HD:(h + 1) * c.HD],
                                     AF.Copy, scale=rtc[:, tt:tt + 1])

        # ---- attention (per batch) + ctx AllGather ----
        ctxbs = [dram.tile([c.DR, c.S], fp8, tag=f"ctxb{l}_{b}",
                           name=f"ctxb{l}_{b}") for b in range(c.B)]
        ctxgs = [dram.tile([c.H, c.S], fp8, addr_space=SHARED,
                           tag=f"ctxg{l}_{b}", name=f"ctxg{l}_{b}")
                 for b in range(c.B)]
        def attn_tail(b, h, cps):
            dr = spool.tile([1, c.S], bf16, tag="dr", name="dr")
            with nc.allow_low_precision(reason="attn 1/den bcast"):
                nc.vector.reciprocal(dr[:], cps[c.HD:c.HD + 1, :])
            bb = psum.tile([c.HD, c.S], f32, tag="ps", name="bb")
            nc.tensor.matmul(bb[:], ones_rb[:, 0:c.HD], dr[:],
                             start=True, stop=True)
            bsb = spool.tile([c.HD, c.S], bf16, tag="bsb",
                             name="bsb", bufs=2)
            nc.vector.tensor_copy(bsb[:], bb[:])
            csb = spool.tile([c.HD, c.S], fp8, tag="csb",
                             name="csb", bufs=2)
            nc.vector.tensor_mul(csb[:], cps[0:c.HD, :], bsb[:])
            nc.sync.dma_start(
                out=ctxbs[b][h * c.HD:(h + 1) * c.HD, :],
                in_=csb[:])

        pend = []
        for b in range(c.B):
            for h0 in range(0, c.HPC, 2):
                hs = [h for h in (h0, h0 + 1) if h < c.HPC]
                cpss = {h: psum.tile([c.HD + 1, c.S], f32, tag="ps",
                                     name=f"cps{h - h0}") for h in hs}
                for t in range(c.SP):
                    pts = {}
                    for h in hs:
                        sps = psum.tile([P, c.S], f32, tag="ps",
                                        name=f"sps{h - h0}")
                        nc.tensor.matmul(
                            sps[:],
                            k_rot[:, h,
                                  b * c.S + t * P:b * c.S + (t + 1) * P],
                            q_rot[:, h, b * c.S:(b + 1) * c.S],
                            start=True, stop=True)
                        pt = ppool.tile([P, c.S], bf16, tag="pt",
                                        name=f"pt{h - h0}")
                        nc.scalar.activation(pt[:], sps[:], AF.Exp,
                                             scale=isqrt_hd)
                        m0 = (c.SP - 1 - t) * P
                        nc.vector.scalar_tensor_tensor(
                            pt[:], pt[:], am2_sb[:, b, t:t + 1],
                            mask_sb[:, m0:m0 + c.S], OP.mult, OP.mult)
                        pts[h] = pt
                    for h in hs:
                        nc.tensor.matmul(
                            cpss[h][:],
                            v_sb[:, b * c.SP + t, h, :],
                            pts[h][:],
                            start=(t == 0), stop=(t == c.SP - 1))
                for bp, hp, cp in pend:
                    attn_tail(bp, hp, cp)
                pend = [(b, h, cpss[h]) for h in hs]
            for bp, hp, cp in pend:
                attn_tail(bp, hp, cp)
            pend = []
            nc.gpsimd.collective_compute(
                "AllGather", OP.bypass, replica_groups=RG,
                ins=[ctxbs[b][:]], outs=[ctxgs[b][:]])

        # ---- o projection per batch + boundary collectives ----
        ln2_ssg = [None] * c.B
        xgo_b = [None] * c.B
        for b in range(c.B):
            ops_b = [psum.tile([P, c.S], f32, tag="ps", name=f"op{b}{ot}")
                     for ot in range(c.OT)]
            for ktp in range(c.KT // 2):
                wt8 = wpool.tile([P, 2, c.OR], fp8, tag="w8",
                                 name="ow8", bufs=3)
                weng = nc.sync if ktp % 2 == 0 else nc.scalar
                weng.dma_start(out=wt8[:], in_=wo[l][ktp])
                ct = xpool.tile([P, 2, c.S], fp8, tag="orhs",
                                name="orhs", bufs=2)
                oeng = nc.scalar if ktp % 2 == 0 else nc.sync
                oeng.dma_start(
                    out=ct[:],
                    in_=ctxgs[b][2 * ktp * P:(2 * ktp + 2) * P, :]
                    .rearrange("(k p) s -> p k s", p=P))
                for ot in range(c.OT):
                    nc.tensor.matmul(
                        ops_b[ot][:], wt8[:, :, ot * P:(ot + 1) * P],
                        ct[:], start=(ktp == 0),
                        stop=(ktp == c.KT // 2 - 1),
                        perf_mode=mybir.MatmulPerfMode.DoubleRow)
            xbo = dram.tile([c.OR, c.S], bf16, tag=f"xbo{l}_{b}",
                            name=f"xbo{l}_{b}")
            xgo = dram.tile([c.H, c.S], bf16, addr_space=SHARED,
                            tag=f"xgo{l}_{b}", name=f"xgo{l}_{b}")
            for ot in range(c.OT):
                xsl = xrows[:, ot, b * c.S:(b + 1) * c.S]
                nc.vector.scalar_tensor_tensor(
                    xsl, ops_b[ot][:], s64_col[:], xsl,
                    OP.mult, OP.add)
                st = xpool.tile([P, c.S], bf16, tag="xst", name="xst",
                                bufs=2)
                nc.scalar.copy(st[:], xsl)
                nc.sync.dma_start(out=xbo[ot * P:(ot + 1) * P, :], in_=st[:])
            ln2_ssg[b] = emit_sumsq_ar(b * c.S, c.S, tag=f"o{l}{b}")
            nc.gpsimd.collective_compute(
                "AllGather", OP.bypass, replica_groups=RG,
                ins=[xbo[:]], outs=[xgo[:]])
            xgo_b[b] = xgo

        # ---- MLP per batch: ln2, gate/up (local), down (input-sharded),
        #      ReduceScatter, residual, sumsq AR, x AllGather ----
        ln1_ssg = [None] * c.B
        xgd_b = [None] * c.B
        rsouts = [None] * c.B
        rsins = [None] * c.B

        it_sbs = [None] * c.B

        def mlp_gu(b):
            emit_norm_post(ln2_ssg[b], xgo_b[b][:], 2 * l + 1, xn,
                           b * c.S, c.S)
            gact = mpool.tile([P, c.FT, c.S], bf16, tag="gact",
                              name=f"gact{l}_{b}", bufs=1)
            it_sb = mpool.tile([P, c.FT, c.S], fp8, tag="it_sb",
                               name=f"it{l}_{b}", bufs=1)
            it_sbs[b] = it_sb
            DR_ = mybir.MatmulPerfMode.DoubleRow
            for phase, wsrc3 in (("g", wg[l]), ("u", wu[l])):
                gps = [psum.tile([P, c.S], f32, tag="ps",
                                 name=f"{phase}{b}_{ot}")
                       for ot in range(c.FT)]
                for ktp in range(c.KT // 2):
                    wt8 = wpool.tile([P, 2, c.FPC], fp8, tag="w8",
                                     name=f"{phase}w8", bufs=3)
                    weng = nc.sync if ktp % 2 == 0 else nc.scalar
                    weng.dma_start(out=wt8[:], in_=wsrc3[ktp])
                    for ot in range(c.FT):
                        nc.tensor.matmul(
                            gps[ot][:],
                            wt8[:, :, ot * P:(ot + 1) * P],
                            xn[:, 2 * ktp:2 * ktp + 2,
                               b * c.S:(b + 1) * c.S],
                            start=(ktp == 0), stop=(ktp == c.KT // 2 - 1),
                            perf_mode=DR_)
                for ot in range(c.FT):
                    if phase == "g":
                        sgt = xpool.tile([P, c.S], bf16, tag="sgt",
                                         name="sgt", bufs=2)
                        nc.scalar.activation(sgt[:], gps[ot][:], AF.Sigmoid,
                                             scale=1.0 / W8SCALE)
                        nc.vector.tensor_mul(
                            gact[:, ot, :], gps[ot][:], sgt[:])
                    else:
                        nc.vector.scalar_tensor_tensor(
                            it_sb[:, ot, :], gps[ot][:], s12_col[:],
                            gact[:, ot, :], OP.mult, OP.mult)

        def mlp_down(b):
            it_sb = it_sbs[b]
            # down: input-sharded over this core's FPC block, out = full H
            rsin = dram.tile([c.H, c.S], bf16, tag=f"rsin{l}_{b}",
                             name=f"rsin{l}_{b}")
            rsout = dram.tile([c.OR, c.S], bf16, tag=f"rsout{l}_{b}",
                              name=f"rsout{l}_{b}")
            DR_ = mybir.MatmulPerfMode.DoubleRow
            for ot in range(c.KT):
                wsb = wpool.tile([P, c.FT // 2, 2, P], fp8, tag="w8",
                                 name="wdw", bufs=3)
                weng = nc.sync if ot % 2 == 0 else nc.scalar
                weng.dma_start(out=wsb[:], in_=wd[l][ot])
                dps = psum.tile([P, c.S], f32, tag="ps", name="dps")
                for ktp in range(c.FT // 2):
                    nc.tensor.matmul(
                        dps[:], wsb[:, ktp, :, :],
                        it_sb[:, 2 * ktp:2 * ktp + 2, :],
                        start=(ktp == 0), stop=(ktp == c.FT // 2 - 1),
                        perf_mode=DR_)
                dp = xpool.tile([P, c.S], bf16, tag="dp", name="dp", bufs=2)
                if ot % 2 == 0:
                    nc.scalar.activation(dp[:], dps[:], AF.Copy,
                                         scale=1.0 / W8SCALE)
                else:
                    nc.vector.tensor_scalar_mul(dp[:], dps[:],
                                                1.0 / W8SCALE)
                nc.sync.dma_start(out=rsin[ot * P:(ot + 1) * P, :],
                                  in_=dp[:])
            rsins[b] = rsin
            rsouts[b] = rsout

        def mlp_rs(b):
            nc.gpsimd.collective_compute(
                "ReduceScatter", OP.add, replica_groups=RG,
                ins=[rsins[b][:]], outs=[rsouts[b][:]])

        def mlp_tail(b):
            """Residual add of the RS output, then x AG (no PE work)."""
            rso = xpool.tile([P, c.OT, c.S], bf16, tag="rso", name="rso",
                             bufs=1)
            for ot in range(c.OT):
                nc.sync.dma_start(out=rso[:, ot, :],
                                  in_=rsouts[b][ot * P:(ot + 1) * P, :])
            xbd = dram.tile([c.OR, c.S], bf16, tag=f"xbd{l}_{b}",
                            name=f"xbd{l}_{b}")
            xgd = dram.tile([c.H, c.S], bf16, addr_space=SHARED,
                            tag=f"xgd{l}_{b}", name=f"xgd{l}_{b}")
            for ot in range(c.OT):
                xsl = xrows[:, ot, b * c.S:(b + 1) * c.S]
                nc.vector.tensor_add(xsl, xsl, rso[:, ot, :])
                st = xpool.tile([P, c.S], bf16, tag="xst", name="xst2",
                                bufs=2)
                nc.scalar.copy(st[:], xsl)
                nc.sync.dma_start(out=xbd[ot * P:(ot + 1) * P, :], in_=st[:])
            nc.gpsimd.collective_compute(
                "AllGather", OP.bypass, replica_groups=RG,
                ins=[xbd[:]], outs=[xgd[:]])
            xgd_b[b] = xgd

        # =====================================================
        # LAYER 1 (slim: k/v full per batch; q/attn/o/MLP last-token)
        # =====================================================
        q_rot2 = persist.tile([c.HD, c.HPC, c.B], bf16, tag="qrot2",
                              name="qrot2")
        k_rot2 = persist.tile([c.HD, c.HPC, c.T], bf16, tag="krot",
                              name="krot2")
        v_sb2 = persist.tile([P, c.TP_, c.HPC, c.HD + 1], bf16, tag="vsb",
                             name="vsb2")
        nc.vector.memset(v_sb2[:, :, :, c.HD:c.HD + 1], 1.0)

        rlastc = persist.tile([c.HD, c.B], f32, tag="rlastc",
                              name="rlastc")
        rlasts = persist.tile([c.HD, c.B], f32, tag="rlasts",
                              name="rlasts")

        def l1_kv(b):
            l = LAST
            bcb = emit_norm_raw_cols(xgd_b[b][:], xn, b * c.S, c.S)
            # rstd-scaled rope tables for this batch + last-token tables
            nc.vector.scalar_tensor_tensor(
                cosb[:, b * c.S:(b + 1) * c.S],
                cos_sb[:, b * c.S:(b + 1) * c.S],
                s64_col[0:c.HD, :], bcb[0:c.HD, :], OP.mult, OP.mult)
            nc.vector.scalar_tensor_tensor(
                sinb[:, b * c.S:(b + 1) * c.S],
                sin_sb[:, b * c.S:(b + 1) * c.S],
                s64_col[0:c.HD, :], bcb[0:c.HD, :], OP.mult, OP.mult)
            nc.vector.scalar_tensor_tensor(
                rlastc[:, b:b + 1], cos2_sb[:, b:b + 1],
                s64_col[0:c.HD, :], bcb[0:c.HD, c.S - 1:c.S],
                OP.mult, OP.mult)
            nc.vector.scalar_tensor_tensor(
                rlasts[:, b:b + 1], sin2_sb[:, b:b + 1],
                s64_col[0:c.HD, :], bcb[0:c.HD, c.S - 1:c.S],
                OP.mult, OP.mult)
            # k pass for this batch
            kg_b = [(h * c.HD, c.HD, c.S, b) for h in range(c.HPC)]
            krhs = (lambda kt, rk, rl, _b=b:
                    xn[:, kt, _b * c.S:(_b + 1) * c.S])
            kps = qk_pair_pass(wk[l], kg_b, f"kp2{b}")
            # rstd column for the v-copy scale (after k: no PE stall)
            rtp = psum.tile([P, c.SP], f32, tag="ps", name=f"rtp2{b}")
            for tt in range(c.SP):
                nc.tensor.matmul(
                    rtp[:, tt:tt + 1],
                    bcb[0:1, tt * P:(tt + 1) * P],
                    id2[0:1, 0:1], start=True, stop=True,
                    is_transpose=True)
            rtc2 = spool.tile([P, c.SP], f32, tag="rtc2", name="rtc2",
                              bufs=2)
            nc.scalar.activation(rtc2[:], rtp[:], AF.Copy,
                                 scale=1.0 / W8SCALE)
            for gi, (c0, cw, n, rk) in enumerate(kg_b):
                h = c0 // c.HD
                emit_rope(kps[gi], k_rot2[:, h, b * c.S:(b + 1) * c.S],
                          cosb[:, b * c.S:(b + 1) * c.S],
                          sinb[:, b * c.S:(b + 1) * c.S], c.S)
            # v pass for this batch
            vps = [psum.tile([P, c.DR], f32, tag="ps", name=f"vp2{b}{tt}")
                   for tt in range(c.SP)]
            DRm = mybir.MatmulPerfMode.DoubleRow
            for ktp in range(c.KT // 2):
                wt = wpool.tile([P, 2, c.DR], fp8, tag="w8", name="vw2")
                weng = nc.sync if ktp % 2 == 0 else nc.scalar
                weng.dma_start(out=wt[:], in_=wv[l][ktp])
                for tt in range(c.SP):
                    gt = b * c.SP + tt
                    nc.tensor.matmul(vps[tt][:],
                                     xn[:, 2 * ktp:2 * ktp + 2,
                                        gt * P:(gt + 1) * P],
                                     wt[:],
                                     start=(ktp == 0),
                                     stop=(ktp == c.KT // 2 - 1),
                                     perf_mode=DRm)
            for tt in range(c.SP):
                for h in range(c.HPC):
                    nc.scalar.activation(
                        v_sb2[:, b * c.SP + tt, h, 0:c.HD],
                        vps[tt][:, h * c.HD:(h + 1) * c.HD],
                        AF.Copy, scale=rtc2[:, tt:tt + 1])

        # interleave: tail(0) hides under gu(1)/down(1); RS(1)+tail(1)
        # hide under layer-1 b0 k/v
        mlp_gu(0)
        mlp_down(0)
        mlp_rs(0)
        mlp_gu(1)
        mlp_down(1)
        mlp_tail(0)
        mlp_rs(1)
        l1_kv(0)
        mlp_tail(1)
        l1_kv(1)
        l = LAST

        # q pass (last tokens only)
        qg2 = [(h * c.HD, c.HD, c.B, 0) for h in range(c.HPC)]
        qps2 = [psum.tile([c.HD, c.B], f32, tag="ps", name=f"qp2{gi}")
                for gi in range(c.HPC)]
        for ktp in range(c.KT // 2):
            wt8 = wpool.tile([P, 2, c.DR], fp8, tag="w8", name="qp2w",
                             bufs=3)
            weng = nc.sync if ktp % 2 == 0 else nc.scalar
            weng.dma_start(out=wt8[:], in_=wq[l][ktp])
            rhs2 = xn[:, 2 * ktp:2 * ktp + 2, :].rearrange(
                "p k (b s) -> p k b s", s=c.S)[:, :, :, c.S - 1]
            for gi, (c0, cw, n, rk) in enumerate(qg2):
                nc.tensor.matmul(
                    qps2[gi][:], wt8[:, :, c0:c0 + cw], rhs2,
                    start=(ktp == 0), stop=(ktp == c.KT // 2 - 1),
                    perf_mode=mybir.MatmulPerfMode.DoubleRow)
        for gi, (c0, cw, n, rk) in enumerate(qg2):
            h = c0 // c.HD
            emit_rope(qps2[gi], q_rot2[:, h, :], rlastc[:], rlasts[:], c.B)

        # ---- replicated last-token state x2 [P, KT, B] fp32 ----
        OTO = c.DR // P
        x2 = persist.tile([P, c.KT, c.B], f32, tag="x2", name="x2")
        x2st = spool.tile([P, c.KT, c.B], bf16, tag="x2st", name="x2st")
        for b in range(c.B):
            nc.sync.dma_start(
                out=x2st[:, :, b],
                in_=xgd_b[b].rearrange("(kt p) s -> p kt s", p=P)
                [:, :, c.S - 1])
        nc.vector.tensor_copy(x2[:], x2st[:])

        def fchunks(total, w=512):
            return [(o, min(w, total - o)) for o in range(0, total, w)]

        def emit_norm_slim_sb(xs, lnidx, dst3):
            """rmsnorm of sbuf fp32 [P, KT, B] -> dst3 bf16."""
            sq = spool.tile([P, c.KT, c.B], f32, tag="sq_slim",
                            name="sq_slim")
            nc.vector.tensor_mul(sq[:], xs[:], xs[:])
            sp_ = psum.tile([1, c.KT * c.B], f32, tag="ps", name="spslim")
            nc.tensor.matmul(sp_[:], ones_c32[:],
                             sq[:].rearrange("p kt b -> p (kt b)"),
                             start=True, stop=True)
            ss2 = spool.tile([1, c.B], f32, tag="ss2", name="ss2")
            nc.vector.tensor_reduce(
                ss2[:], sp_[:].rearrange("o (kt b) -> o b kt", b=c.B),
                mybir.AxisListType.X, OP.add)
            bc = emit_rstd_bcast([ss2[:]], c.B, [(0, c.B)])
            tmp = spool.tile([P, c.KT, c.B], f32, tag="tmp_slim",
                             name="tmp_slim")
            nc.vector.tensor_tensor(
                tmp[:], xs[:],
                lnw_sb[:, lnidx, :].unsqueeze(2).broadcast_to(
                    (P, c.KT, c.B)), OP.mult)
            nc.vector.tensor_tensor(
                dst3[:], tmp[:],
                bc[:].unsqueeze(1).broadcast_to((P, c.KT, c.B)), OP.mult)

        def emit_rstd_col(xs, name):
            """fp32 [P, KT, B] -> [B, 1] rsqrt(mean+eps) column (PE work
            is two tiny ops; scalar chain runs in parallel)."""
            sq = spool.tile([P, c.KT, c.B], f32, tag="sq_slim",
                            name=f"sqr{name}")
            nc.vector.tensor_mul(sq[:], xs[:], xs[:])
            sp_ = psum.tile([1, c.KT * c.B], f32, tag="ps",
                            name=f"sp{name}")
            nc.tensor.matmul(sp_[:], ones_c32[:],
                             sq[:].rearrange("p kt b -> p (kt b)"),
                             start=True, stop=True)
            ss2 = spool.tile([1, c.B], f32, tag="ss2", name=f"ss{name}")
            nc.vector.tensor_reduce(
                ss2[:], sp_[:].rearrange("o (kt b) -> o b kt", b=c.B),
                mybir.AxisListType.X, OP.add)
            lt = spool.tile([1, c.B], f32, tag="lt", name=f"lt{name}")
            nc.scalar.activation(lt[:], ss2[:], AF.Ln,
                                 bias=eps_col[0:1, :], scale=1.0 / c.H)
            rt = spool.tile([1, c.B], f32, tag="rt", name=f"rt{name}")
            nc.scalar.activation(rt[:], lt[:], AF.Exp, scale=-0.5)
            rcp = psum.tile([c.B, 1], f32, tag="ps", name=f"rc{name}")
            nc.tensor.matmul(rcp[:], rt[:], id2[0:1, 0:1],
                             start=True, stop=True, is_transpose=True)
            rc = spool.tile([c.B, 1], f32, tag="rc", name=f"rcc{name}",
                            bufs=2)
            nc.scalar.copy(rc[:], rcp[:])
            return rc

        def emit_slim_ar(src_fn, nchunks_w, arname, wsrc_fn, kts, lhsT_sb,
                         pscale=1.0):
            """Token-major projection out[B, H] = lhsT.T @ W, AllReduce,
            and return a [P, KT, B] stage tile of the result."""
            arin = dram.tile([c.B, c.H], bf16, tag=f"arin{arname}",
                             name=f"arin{arname}")
            arout = dram.tile([c.B, c.H], bf16, addr_space=SHARED,
                              tag=f"arout{arname}", name=f"arout{arname}")
            ocs = fchunks(c.H)
            psl = [psum.tile([c.B, cw], f32, tag="ps",
                             name=f"{arname}ps{oc}")
                   for oc, (c0, cw) in enumerate(ocs)]
            di = 0
            for kt in range(kts):
                for h0, hw in fchunks(c.H, 1536):
                    wt = wpool.tile([P, 1536], fp8, tag="w8",
                                    name=f"{arname}w", bufs=3)
                    eng = nc.sync if di % 2 == 0 else nc.scalar
                    di += 1
                    eng.dma_start(out=wt[:, 0:hw],
                                  in_=wsrc_fn(kt)[:, h0:h0 + hw])
                    for oc, (c0, cw) in enumerate(ocs):
                        if c0 < h0 or c0 >= h0 + hw:
                            continue
                        nc.tensor.matmul(psl[oc][:], lhsT_sb(kt),
                                         wt[:, c0 - h0:c0 - h0 + cw],
                                         start=(kt == 0),
                                         stop=(kt == kts - 1))
            for oc, (c0, cw) in enumerate(ocs):
                osl = spool.tile([c.B, 512], bf16, tag="osl", name="osl",
                                 bufs=2)
                nc.scalar.activation(osl[:, 0:cw], psl[oc][:], AF.Copy,
                                     scale=pscale)
                nc.sync.dma_start(out=arin[:, c0:c0 + cw],
                                  in_=osl[:, 0:cw])
            nc.gpsimd.collective_compute(
                "AllReduce", OP.add, replica_groups=RG,
                ins=[arin[:]], outs=[arout[:]])
            stage = spool.tile([P, c.KT, c.B], bf16, tag="arstage",
                               name=f"arst{arname}", bufs=2)
            for t in range(c.B):
                nc.sync.dma_start(
                    out=stage[:, :, t],
                    in_=arout[t, :].rearrange("(kt p) -> p kt", p=P))
            return stage

        # ---- slim attention: all 8 units interleaved, packed psums ----
        cpk = persist.tile([P, OTO, c.B], bf16, tag="cpk", name="cpk")
        NU = c.B * c.HPC
        spsA = psum.tile([P, NU, c.SP], f32, tag="ps", name="spsA")
        for u in range(NU):
            b, h = u // c.HPC, u % c.HPC
            for t in range(c.SP):
                nc.tensor.matmul(
                    spsA[:, u, t:t + 1],
                    k_rot2[:, h, b * c.S + t * P:b * c.S + (t + 1) * P],
                    q_rot2[:, h, b:b + 1],
                    start=True, stop=True)
        ptA = ppool.tile([P, NU, c.SP], bf16, tag="ptA", name="ptA")
        for u in range(NU):
            b = u // c.HPC
            nc.scalar.activation(ptA[:, u, :], spsA[:, u, :], AF.Exp,
                                 scale=isqrt_hd)
            nc.vector.tensor_mul(ptA[:, u, :], ptA[:, u, :],
                                 am2_sb[:, b, :])
        cpsA = psum.tile([c.HD + 1, NU], f32, tag="ps", name="cpsA")
        for u in range(NU):
            b, h = u // c.HPC, u % c.HPC
            for t in range(c.SP):
                nc.tensor.matmul(
                    cpsA[:, u:u + 1],
                    v_sb2[:, b * c.SP + t, h, :],
                    ptA[:, u, t:t + 1],
                    start=(t == 0), stop=(t == c.SP - 1))
        rA = spool.tile([1, NU], f32, tag="rA", name="rA")
        nc.vector.reciprocal(rA[:], cpsA[c.HD:c.HD + 1, :])
        bbA = psum.tile([c.HD, NU], f32, tag="ps", name="bbA")
        nc.tensor.matmul(bbA[:], ones_r32[:, 0:c.HD], rA[:],
                         start=True, stop=True)
        bsbA = spool.tile([c.HD, NU], f32, tag="bsbA", name="bsbA")
        nc.vector.tensor_copy(bsbA[:], bbA[:])
        csbA = spool.tile([c.HD, NU], bf16, tag="csbA", name="csbA")
        nc.vector.tensor_mul(csbA[:], cpsA[0:c.HD, :], bsbA[:])
        for u in range(NU):
            b, h = u // c.HPC, u % c.HPC
            f0, srcp = h * c.HD, 0
            rem = c.HD
            while rem > 0:
                kt, po = f0 // P, f0 % P
                n = min(P - po, rem)
                nc.sync.dma_start(
                    out=cpk[po:po + n, kt, b:b + 1],
                    in_=csbA[srcp:srcp + n, u:u + 1])
                f0 += n
                srcp += n
                rem -= n

        # ---- slim o projection: token-major partial + AllReduce ----
        ost = emit_slim_ar(None, None, "o",
                           lambda kt: wo2s[kt], OTO,
                           lambda kt: cpk[:, kt, :],
                           pscale=1.0 / W8SCALE)
        nc.vector.tensor_add(x2[:], x2[:], ost[:])

        # ---- slim ln2 (rstd folded into sigmoid/down scales) + MLP ----
        xn2 = persist.tile([P, c.KT, c.B], bf16, tag="xn2", name="xn2")
        nc.vector.tensor_copy(xn2[:], x2[:])
        rc2 = emit_rstd_col(x2, "n2")

        FC = fchunks(c.FPC)
        gps2 = [psum.tile([c.B, cw], f32, tag="ps", name=f"g2_{j}")
                for j, (c0, cw) in enumerate(FC)]
        ups2 = [psum.tile([c.B, cw], f32, tag="ps", name=f"u2_{j}")
                for j, (c0, cw) in enumerate(FC)]
        for kt in range(c.KT):
            wgut = wpool.tile([P, 2 * c.FPC], fp8, tag="w8", name="wguw",
                              bufs=3)
            weng = nc.scalar if (kt < 3 or kt % 2 == 1) else nc.sync
            weng.dma_start(out=wgut[:], in_=wgu1[kt])
            for j, (c0, cw) in enumerate(FC):
                nc.tensor.matmul(gps2[j][:], xn2[:, kt, :],
                                 wgut[:, c0:c0 + cw],
                                 start=(kt == 0), stop=(kt == c.KT - 1))
            for j, (c0, cw) in enumerate(FC):
                nc.tensor.matmul(ups2[j][:], xn2[:, kt, :],
                                 wgut[:, c.FPC + c0:c.FPC + c0 + cw],
                                 start=(kt == 0), stop=(kt == c.KT - 1))
        rsig = spool.tile([c.B, 1], f32, tag="rsig", name="rsig")
        nc.vector.tensor_scalar_mul(rsig[:], rc2[:], 1.0 / W8SCALE)
        rdwn = spool.tile([c.B, 1], f32, tag="rdwn", name="rdwn")
        nc.vector.tensor_mul(rdwn[:], rc2[:], rc2[:])
        nc.vector.tensor_scalar_mul(rdwn[:], rdwn[:], 1.0 / W8SCALE ** 3)
        it2 = spool.tile([c.B, c.FPC], bf16, tag="it2", name="it2")
        for j, (c0, cw) in enumerate(FC):
            sg2 = spool.tile([c.B, 512], bf16, tag="sg2", name="sg2",
                             bufs=2)
            nc.scalar.activation(sg2[:, 0:cw], gps2[j][:], AF.Sigmoid,
                                 scale=rsig[:])
            ga2 = spool.tile([c.B, 512], bf16, tag="ga2", name="ga2",
                             bufs=2)
            nc.vector.tensor_mul(ga2[:, 0:cw], gps2[j][:], sg2[:, 0:cw])
            nc.vector.tensor_mul(it2[:, c0:c0 + cw], ups2[j][:],
                                 ga2[:, 0:cw])
        # transpose int [B, FPC] -> [P, FT, B] via PE
        intp = psum.tile([P, c.FT, c.B], bf16, tag="ps", name="intp")
        for j2 in range(c.FT):
            nc.tensor.matmul(intp[:, j2, :], it2[:, j2 * P:(j2 + 1) * P],
                             id2b[:], start=True, stop=True,
                             is_transpose=True)
        intT = spool.tile([P, c.FT, c.B], bf16, tag="intT", name="intT")
        nc.vector.tensor_copy(intT[:], intp[:])

        # ---- slim down: token-major partial + AllReduce ----
        dst_ = emit_slim_ar(None, None, "d",
                            lambda kt: wd[l][kt], c.FT,
                            lambda kt: intT[:, kt, :],
                            pscale=rdwn[:])
        nc.vector.tensor_add(x2[:], x2[:], dst_[:])

        # ================= final norm (folded) + cls head =================
        xnf = persist.tile([P, c.KT, c.B], bf16, tag="xnf", name="xnf")
        nc.vector.tensor_copy(xnf[:], x2[:])
        rc3 = emit_rstd_col(x2, "nf")

        CC1 = fchunks(c.CLS)
        hps = [psum.tile([c.B, cw], f32, tag="ps", name=f"hps{j}")
               for j, (c0, cw) in enumerate(CC1)]
        for kt in range(c.KT):
            wt = wpool.tile([P, c.CLS], bf16, tag="wt", name="w1w",
                            bufs=3)
            nc.sync.dma_start(out=wt[:], in_=w1t[kt])
            for j, (c0, cw) in enumerate(CC1):
                nc.tensor.matmul(hps[j][:], xnf[:, kt, :],
                                 wt[:, c0:c0 + cw],
                                 start=(kt == 0), stop=False)
        binv = spool.tile([c.B, 1], f32, tag="binv", name="binv")
        nc.vector.reciprocal(binv[:], rc3[:])
        bivp = psum.tile([1, c.B], f32, tag="ps", name="bivp")
        nc.tensor.matmul(bivp[:], binv[:], id2[:], start=True, stop=True,
                         is_transpose=True)
        binr = spool.tile([1, c.B], bf16, tag="binr", name="binr")
        nc.scalar.copy(binr[:], bivp[:])
        for j, (c0, cw) in enumerate(CC1):
            nc.tensor.matmul(hps[j][:], binr[:],
                             b1_sb[:, c0:c0 + cw],
                             start=False, stop=True)
        h2 = spool.tile([c.B, c.CLS], bf16, tag="h2", name="h2")
        for j, (c0, cw) in enumerate(CC1):
            nc.scalar.activation(h2[:, c0:c0 + cw], hps[j][:], AF.Relu,
                                 scale=rc3[:])
        hq2 = spool.tile([c.B, c.CLS], f32, tag="hq2", name="hq2")
        nc.vector.tensor_mul(hq2[:], h2[:], h2[:])
        mrow = spool.tile([c.B, 1], f32, tag="mrow", name="mrow")
        nc.vector.tensor_reduce(mrow[:], h2[:], mybir.AxisListType.X,
                                OP.add)
        srow2 = spool.tile([c.B, 1], f32, tag="srow2", name="srow2")
        nc.vector.tensor_reduce(srow2[:], hq2[:], mybir.AxisListType.X,
                                OP.add)
        m_sb = spool.tile([c.B, 1], f32, tag="m_sb", name="m_sb")
        nc.vector.tensor_scalar_mul(m_sb[:], mrow[:], 1.0 / c.CLS)
        s_sb = spool.tile([c.B, 1], f32, tag="s_sb", name="s_sb")
        nc.vector.tensor_scalar_mul(s_sb[:], srow2[:], 1.0 / c.CLS)
        msq = spool.tile([c.B, 1], f32, tag="msq", name="msq")
        nc.vector.tensor_mul(msq[:], m_sb[:], m_sb[:])
        var = spool.tile([c.B, 1], f32, tag="var", name="var")
        nc.vector.tensor_sub(var[:], s_sb[:], msq[:])
        lv = spool.tile([c.B, 1], f32, tag="lv", name="lv")
        nc.scalar.activation(lv[:], var[:], AF.Ln, bias=eps_col[0:c.B, :])
        rstd = spool.tile([c.B, 1], f32, tag="rstd", name="rstd")
        nc.scalar.activation(rstd[:], lv[:], AF.Exp, scale=-0.5)
        hn = spool.tile([c.B, c.CLS], bf16, tag="hn", name="hn")
        nc.vector.tensor_scalar(hn[:], h2[:], m_sb[:], rstd[:],
                                OP.subtract, OP.mult)
        # transpose hn [B, CLS] -> [P, CT, B], then logits
        hTp = psum.tile([P, c.CT, c.B], bf16, tag="ps", name="hTp")
        for j2 in range(c.CT):
            nc.tensor.matmul(hTp[:, j2, :], hn[:, j2 * P:(j2 + 1) * P],
                             id2b[:], start=True, stop=True,
                             is_transpose=True)
        hT = spool.tile([P, c.CT, c.B], bf16, tag="hT", name="hT")
        nc.vector.tensor_copy(hT[:], hTp[:])
        w2w = wpool.tile([P, c.CT, c.NCLS], bf16, tag="w2w", name="w2w")
        nc.sync.dma_start(out=w2w[:],
                          in_=w2g[:].rearrange("g p m -> p g m"))
        lg = psum.tile([c.B, c.NCLS], f32, tag="ps", name="lg")
        for j2 in range(c.CT):
            nc.tensor.matmul(lg[:], hT[:, j2, :], w2w[:, j2, :],
                             start=(j2 == 0), stop=False)
        nc.tensor.matmul(lg[:], ones_r2[:, 0:c.B], b2_sb[:],
                         start=False, stop=True)
        lg_sb = spool.tile([c.B, c.NCLS], f32, tag="lg_sb", name="lg_sb")
        nc.vector.tensor_copy(lg_sb[:], lg[:])
        nc.sync.dma_start(out=out_d.rearrange("cc b -> b cc"),
                          in_=lg_sb[:])

    nc.compile()
    return nc


# ----------------------------------------------------------------------------
# entry point
# ----------------------------------------------------------------------------

_CACHE = {}


def _get_nc(cfg):
    if cfg not in _CACHE:
        _CACHE[cfg] = build_nc(cfg)
    return _CACHE[cfg]


def run(cfg, inputs, trace=False, **kw):
    from concourse.bass_utils import run_bass_kernel_spmd
    in_maps = host_prep(cfg, inputs)
    nc = _get_nc(cfg)
    res = run_bass_kernel_spmd(nc, in_maps, core_ids=list(range(cfg.NC)),
                               trace=trace, **kw)
    out = np.asarray(res.results[0]["logits_out"])  # [NCLS, B]
    return np.ascontiguousarray(out.T.astype(np.float32)), res


def kernel(**inputs):
    inputs = {k: np.asarray(v) for k, v in inputs.items()}
    out, _ = run(FULL_CFG, inputs)
    return out


# revision 55
# speedup vs baseline: 1.0091x; 1.0091x over previous
"""Trainium2 Bass kernel for nn_Net_89361089561102 (2-layer dense transformer,
NF4-quantized weights, cls head). Tensor-parallel over 8 NeuronCores.

Strategy (v2):
 - Host: unpack NF4 weights -> bf16, pre-transpose to [K, M] layout, shard
   TP-style (qkv/gate_up by output dim == heads/ff, o by output dim, down
   INPUT-sharded by ff), embedding gather, RoPE cos/sin tables, causal masks.
 - Device layer 0 (full tokens), per-batch pipelined so collectives hide
   under the other batch's compute:
     ln1 (local) -> qkv + RoPE -> attention -> per-batch: [ctx AllGather,
     o_proj (output-sharded), residual, sumsq AllReduce, x AllGather,
     ln2, gate/up (local intermediate kept in SBUF), down (INPUT-sharded,
     no intermediate AllGather), ReduceScatter, residual, sumsq AllReduce,
     x AllGather].
 - Layer 1 (last layer): k/v full (per-batch passes so they start as soon
   as that batch's x AllGather lands); q/attn/o/MLP only at the last token
   of each batch; final rmsnorm + cls head redundant on every core.
"""

import math
from contextlib import ExitStack
from dataclasses import dataclass

import numpy as np
import ml_dtypes

BF16 = ml_dtypes.bfloat16
FP8 = ml_dtypes.float8_e4m3
W8SCALE = 64.0
EPS = 1e-5
BLK = 64
NF4 = np.array([
    -1.0, -0.6961928009986877, -0.5250730514526367, -0.39491748809814453,
    -0.28444138169288635, -0.18477343022823334, -0.09105003625154495, 0.0,
    0.07958029955625534, 0.16093020141124725, 0.24611230194568634,
    0.33791524171829224, 0.44070982933044434, 0.5626170039176941,
    0.7229568362236023, 1.0], dtype=np.float32)


@dataclass(frozen=True)
class Cfg:
    H: int
    NH: int
    HD: int
    FF: int
    B: int
    S: int
    L: int
    NC: int
    CLS: int = 768
    NCLS: int = 2
    P: int = 128

    @property
    def T(self):
        return self.B * self.S

    @property
    def KT(self):
        return self.H // self.P

    @property
    def KTF(self):
        return self.FF // self.P

    @property
    def HPC(self):  # heads per core
        return self.NH // self.NC

    @property
    def DR(self):  # q/k/v rows per core
        return self.HPC * self.HD

    @property
    def OR(self):  # o/down output rows per core
        return self.H // self.NC

    @property
    def OT(self):
        return self.OR // self.P

    @property
    def FPC(self):  # ff rows per core
        return self.FF // self.NC

    @property
    def FT(self):
        return self.FPC // self.P

    @property
    def SP(self):  # seq tiles per batch
        return self.S // self.P

    @property
    def TP_(self):  # token tiles total
        return self.T // self.P

    @property
    def CT(self):
        return self.CLS // self.P

    def check(self):
        assert self.H % self.P == 0 and self.FF % self.P == 0
        assert self.S % self.P == 0 and self.S <= 512
        assert self.NH % self.NC == 0 and self.H % self.NC == 0
        assert self.FF % self.NC == 0
        assert self.OR % self.P == 0 and self.FPC % self.P == 0
        assert self.HD <= self.P and self.HD % 2 == 0
        assert self.HPC * self.B <= 8  # q/k psum groups
        assert self.SP <= 8            # v psum groups (per batch)
        assert self.OT <= 8            # o psum groups (per batch)
        assert self.FT <= 8            # gate/up psum groups (per batch)
        assert self.CLS % self.P == 0


FULL_CFG = Cfg(H=3072, NH=32, HD=96, FF=8192, B=2, S=512, L=2, NC=8)


# ----------------------------------------------------------------------------
# host-side prep
# ----------------------------------------------------------------------------

def dequant_np(packed, absmax, out_f, in_f):
    shifts = (np.arange(8, dtype=np.int32) * 4)
    codes = ((packed[:, None] >> shifts) & 0xF).reshape(-1)
    w = (NF4[codes].reshape(-1, BLK) * absmax[:, None].astype(np.float32))
    return w.reshape(out_f, in_f)


def _wt3(w_t, P):
    """[K, M] fp32 -> [K//P, P, M] bf16 contiguous."""
    K, M = w_t.shape
    return np.ascontiguousarray(w_t.reshape(K // P, P, M).astype(BF16))


def host_prep(cfg: Cfg, inputs):
    """Full inputs -> list of per-core input maps."""
    c = cfg
    P = c.P
    x = inputs["embed"][inputs["input_ids"]]          # [B, S, H] fp32
    x0f = np.ascontiguousarray(x.reshape(c.T, c.H).T.astype(np.float32))
    x0 = np.ascontiguousarray(x0f.astype(BF16))        # [H, T] bf16

    # rope tables
    inv = 1.0 / (10000.0 ** (np.arange(0, c.HD, 2, dtype=np.float32) / c.HD))
    f = np.outer(np.arange(c.S, dtype=np.float32), inv)
    emb = np.concatenate([f, f], -1)                   # [S, HD]
    sgn = np.concatenate([-np.ones(c.HD // 2, np.float32),
                          np.ones(c.HD // 2, np.float32)])
    cosT = np.tile(np.cos(emb).T, (1, c.B))            # [HD, T]
    sinT = np.tile(np.sin(emb).T * sgn[:, None], (1, c.B))
    last = np.array([b * c.S + c.S - 1 for b in range(c.B)])
    cosT2 = np.ascontiguousarray(cosT[:, last].astype(np.float32))
    sinT2 = np.ascontiguousarray(sinT[:, last].astype(np.float32))
    cosT = np.ascontiguousarray(cosT.astype(BF16))
    sinT = np.ascontiguousarray(sinT.astype(BF16))

    am = (inputs["attention_mask"] != 0)               # [B, S]
    tk = np.arange(c.S)
    EW = c.S + (c.SP - 1) * P
    jj = np.arange(EW)
    m1 = (np.arange(P)[:, None] <= jj[None, :] - (c.SP - 1) * P)
    m1 = np.ascontiguousarray(m1.astype(BF16))
    am2 = np.zeros((c.B, P, c.SP), np.float32)
    for b in range(c.B):
        am2[b] = am[b].reshape(c.SP, P).T
    am2 = am2.astype(BF16)

    # layernorm weights [5, P, KT]
    lnw = np.zeros((2 * c.L + 1, P, c.KT), np.float32)
    for l in range(c.L):
        lnw[2 * l] = inputs["ln1_w"][l].reshape(c.KT, P).T
        lnw[2 * l + 1] = inputs["ln2_w"][l].reshape(c.KT, P).T
    lnw[2 * c.L] = inputs["final_ln_w"].reshape(c.KT, P).T

    # cls head: fold ln_g into w2, ln_b@w2.T+b2 into the output bias
    w1t = _wt3(inputs["w1"].astype(np.float32).T
               * inputs["final_ln_w"].astype(np.float32)[:, None],
               P)                                          # [KT, P, CLS]
    b1row = np.ascontiguousarray(
        inputs["b1"].reshape(1, c.CLS).astype(BF16))
    w2_f = inputs["w2"].astype(np.float32)                 # [NCLS, CLS]
    w2g = np.ascontiguousarray(
        (w2_f * inputs["ln_g"][None, :]).T.reshape(c.CT, P, c.NCLS)
        .astype(BF16))
    b2row = np.ascontiguousarray(
        (inputs["ln_b"].astype(np.float32) @ w2_f.T
         + inputs["b2"].astype(np.float32)).reshape(1, c.NCLS))

    shared = dict(x0=x0, cosT=cosT, sinT=sinT, cosT2=cosT2, sinT2=sinT2,
                  m1=m1, am2=am2, lnw=lnw, w1t=w1t, b1row=b1row, w2g=w2g,
                  b2row=b2row, ident2=np.eye(2, dtype=np.float32))

    # per-layer dequantized weights (full) then shard
    per_layer = []
    for l in range(c.L):
        wqkv = dequant_np(inputs["qkv_packed"][l], inputs["qkv_absmax"][l],
                          3 * c.H, c.H)
        wo = dequant_np(inputs["o_packed"][l], inputs["o_absmax"][l],
                        c.H, c.H)
        wgu = dequant_np(inputs["gu_packed"][l], inputs["gu_absmax"][l],
                         2 * c.FF, c.H)
        wd = dequant_np(inputs["down_packed"][l], inputs["down_absmax"][l],
                        c.H, c.FF)
        per_layer.append((wqkv, wo, wgu, wd))

    in_maps = []
    for core in range(c.NC):
        m = dict(shared)
        m["x0r"] = np.ascontiguousarray(
            x0f[core * c.OR:(core + 1) * c.OR, :])
        for l in range(c.L):
            wqkv, wo, wgu, wd = per_layer[l]
            d0 = core * c.DR
            lf = inputs["ln1_w"][l].astype(np.float32)[None, :]

            def _pair8(w_t):
                K8, M8 = w_t.shape
                return np.ascontiguousarray(
                    (w_t * W8SCALE).reshape(K8 // (2 * P), 2, P, M8)
                    .transpose(0, 2, 1, 3).astype(FP8))
            m[f"wq{l}"] = _pair8((wqkv[d0:d0 + c.DR, :] * lf).T)
            m[f"wk{l}"] = _pair8((wqkv[c.H + d0:c.H + d0 + c.DR, :] * lf).T)
            m[f"wv{l}"] = _pair8(
                (wqkv[2 * c.H + d0:2 * c.H + d0 + c.DR, :] * lf).T)
            o0 = core * c.OR
            m[f"wo{l}"] = _pair8(wo[o0:o0 + c.OR, :].T)
            g0 = core * c.FPC

            def _wt8p(w_t):
                # [H, M] -> [H/2P, P, 2, M] fp8 (DoubleRow pair layout)
                K, M = w_t.shape
                arr = (w_t * W8SCALE).reshape(K // (2 * P), 2, P, M)
                return np.ascontiguousarray(
                    arr.transpose(0, 2, 1, 3).astype(FP8))
            if l == c.L - 1:
                m[f"wg{l}"] = _wt3(wgu[g0:g0 + c.FPC, :].T, P)
                m[f"wu{l}"] = _wt3(wgu[c.FF + g0:c.FF + g0 + c.FPC, :].T, P)
            else:
                m[f"wg{l}"] = _wt8p(wgu[g0:g0 + c.FPC, :].T)
                m[f"wu{l}"] = _wt8p(wgu[c.FF + g0:c.FF + g0 + c.FPC, :].T)
            if l == c.L - 1:
                # slim layer: INPUT-sharded down [FT, P, H] and o [DR/P, P, H]
                # stored fp8 (x W8SCALE) to halve the weight stream
                def _wt8(w_t):
                    K, M = w_t.shape
                    return np.ascontiguousarray(
                        (w_t * W8SCALE).reshape(K // P, P, M).astype(FP8))
                wdt = np.ascontiguousarray(wd[:, g0:g0 + c.FPC].T)
                m[f"wd{l}"] = _wt8(wdt)
                wot = np.ascontiguousarray(wo[:, d0:d0 + c.DR].T)
                m["wo2s"] = _wt8(wot)
                ln2c = inputs["ln2_w"][l].astype(np.float32).reshape(
                    c.KT, P, 1)
                m["wgu1"] = np.ascontiguousarray(np.concatenate(
                    [m[f"wg{l}"].astype(np.float32),
                     m[f"wu{l}"].astype(np.float32)], axis=2)
                    * ln2c * W8SCALE).astype(FP8)
                del m[f"wg{l}"], m[f"wu{l}"]
            else:
                # full layer: INPUT-sharded down, ot-major DoubleRow pairs
                # [OT=KT, P, FT/2, 2, P] fp8 (x W8SCALE)
                wdt = np.ascontiguousarray(wd[:, g0:g0 + c.FPC].T)  # [FPC, H]
                arr = (wdt * W8SCALE).reshape(c.FT // 2, 2, P, c.KT, P)
                arr = arr.transpose(3, 2, 0, 1, 4)
                m[f"wd{l}"] = np.ascontiguousarray(arr.astype(FP8))
        in_maps.append(m)
    return in_maps


# ----------------------------------------------------------------------------
# device kernel
# ----------------------------------------------------------------------------

def build_nc(cfg: Cfg):
    import concourse.bass as bass
    import concourse.mybir as mybir
    import concourse.tile as tile
    from concourse import bacc

    c = cfg
    c.check()
    P = c.P
    f32 = mybir.dt.float32
    bf16 = mybir.dt.bfloat16
    AF = mybir.ActivationFunctionType
    OP = mybir.AluOpType

    nc = bacc.Bacc("TRN2", target_bir_lowering=False, debug=False,
                   enable_asserts=False, num_devices=c.NC)
    RG = [list(range(c.NC))]
    SHARED = "Shared" if c.NC > 4 else "Local"

    def din(name, shape, dt):
        return nc.dram_tensor(name, list(shape), dt, kind="ExternalInput").ap()

    x0 = din("x0", [c.H, c.T], bf16)
    x0r = din("x0r", [c.OR, c.T], f32)
    cosT = din("cosT", [c.HD, c.T], bf16)
    sinT = din("sinT", [c.HD, c.T], bf16)
    cosT2 = din("cosT2", [c.HD, c.B], f32)
    sinT2 = din("sinT2", [c.HD, c.B], f32)
    m1 = din("m1", [P, c.S + (c.SP - 1) * P], bf16)
    am2 = din("am2", [c.B, P, c.SP], bf16)
    lnw_d = din("lnw", [2 * c.L + 1, P, c.KT], f32)
    w1t = din("w1t", [c.KT, P, c.CLS], bf16)
    b1r = din("b1row", [1, c.CLS], bf16)
    w2g = din("w2g", [c.CT, P, c.NCLS], bf16)
    b2r = din("b2row", [1, c.NCLS], f32)
    ident2 = din("ident2", [2, 2], f32)
    fp8 = mybir.dt.float8e4
    wq = [din(f"wq{l}", [c.KT // 2, P, 2, c.DR], fp8) for l in range(c.L)]
    wk = [din(f"wk{l}", [c.KT // 2, P, 2, c.DR], fp8) for l in range(c.L)]
    wv = [din(f"wv{l}", [c.KT // 2, P, 2, c.DR], fp8)
          for l in range(c.L)]
    wo = [din(f"wo{l}", [c.KT // 2, P, 2, c.OR], fp8)
          for l in range(c.L)]
    wg = [din(f"wg{l}", [c.KT // 2, P, 2, c.FPC], fp8)
          for l in range(c.L - 1)]
    wu = [din(f"wu{l}", [c.KT // 2, P, 2, c.FPC], fp8)
          for l in range(c.L - 1)]
    wgu1 = din("wgu1", [c.KT, P, 2 * c.FPC], fp8)
    wd = [din(f"wd{l}",
              [c.FT, P, c.H] if l == c.L - 1 else
              [c.KT, P, c.FT // 2, 2, P],
              fp8) for l in range(c.L)]
    wo2s = din("wo2s", [c.DR // P, P, c.H], fp8)
    out_d = nc.dram_tensor("logits_out", [c.NCLS, c.B], f32,
                           kind="ExternalOutput").ap()

    isqrt_hd = 1.0 / math.sqrt(c.HD)
    LAST = c.L - 1

    def lastcols(ap2d):
        """[P, T] AP -> [P, B] AP selecting the last token of each batch."""
        return ap2d.rearrange("p (b s) -> p b s", s=c.S)[:, :, c.S - 1]

    with tile.TileContext(nc) as tc, ExitStack() as ctx:
        const = ctx.enter_context(tc.tile_pool(name="const", bufs=1))
        persist = ctx.enter_context(tc.tile_pool(name="persist", bufs=1))
        wpool = ctx.enter_context(tc.tile_pool(name="wpool", bufs=3))
        xpool = ctx.enter_context(tc.tile_pool(name="xpool", bufs=3))
        spool = ctx.enter_context(tc.tile_pool(name="spool", bufs=2))
        ppool = ctx.enter_context(tc.tile_pool(name="ppool", bufs=3))
        rpool = ctx.enter_context(tc.tile_pool(name="rpool", bufs=1))
        mpool = ctx.enter_context(tc.tile_pool(name="mpool", bufs=2))
        psum = ctx.enter_context(tc.tile_pool(name="psum", bufs=8,
                                              space="PSUM"))
        dram = ctx.enter_context(tc.tile_pool(name="dram", bufs=1,
                                              space="DRAM"))

        # ---- constants in SBUF ----
        ones_c32 = const.tile([P, 1], f32, tag="ones_c32")
        nc.vector.memset(ones_c32[:], 1.0)
        ones_cbf = const.tile([P, 1], bf16, tag="ones_cbf")
        nc.vector.memset(ones_cbf[:], 1.0)
        ones_r32 = const.tile([1, P], f32, tag="ones_r32")
        nc.vector.memset(ones_r32[:], 1.0)
        eps_col = const.tile([P, 1], f32, tag="eps_col")
        nc.vector.memset(eps_col[:], EPS)
        cos_sb = const.tile([c.HD, c.T], bf16, tag="cos_sb")
        nc.scalar.dma_start(out=cos_sb[:], in_=cosT)
        sin_sb = const.tile([c.HD, c.T], bf16, tag="sin_sb")
        nc.scalar.dma_start(out=sin_sb[:], in_=sinT)
        cos2_sb = const.tile([c.HD, c.B], f32, tag="cos2_sb")
        nc.sync.dma_start(out=cos2_sb[:], in_=cosT2)
        sin2_sb = const.tile([c.HD, c.B], f32, tag="sin2_sb")
        nc.sync.dma_start(out=sin2_sb[:], in_=sinT2)
        mask_sb = const.tile([P, c.S + (c.SP - 1) * P], bf16,
                             tag="mask_sb")
        nc.scalar.dma_start(out=mask_sb[:], in_=m1)
        am2_sb = const.tile([P, c.B, c.SP], bf16, tag="am2_sb")
        for b in range(c.B):
            nc.sync.dma_start(out=am2_sb[:, b, :], in_=am2[b])
        lnw_sb = const.tile([P, 2 * c.L + 1, c.KT], f32, tag="lnw_sb")
        for n in range(2 * c.L + 1):
            nc.sync.dma_start(out=lnw_sb[:, n, :], in_=lnw_d[n])
        b1_sb = const.tile([1, c.CLS], bf16, tag="b1_sb")
        nc.sync.dma_start(out=b1_sb[:], in_=b1r)
        b2_sb = const.tile([1, c.NCLS], f32, tag="b2_sb")
        nc.sync.dma_start(out=b2_sb[:], in_=b2r)
        ones_rbf = const.tile([1, 2], bf16, tag="ones_rbf")
        nc.vector.memset(ones_rbf[:], 1.0)
        ones_rb = const.tile([1, P], bf16, tag="ones_rb")
        nc.vector.memset(ones_rb[:], 1.0)
        ones_r2 = const.tile([1, 2], f32, tag="ones_r2")
        nc.vector.memset(ones_r2[:], 1.0)
        id2 = const.tile([2, 2], f32, tag="id2")
        nc.sync.dma_start(out=id2[:], in_=ident2)
        id2b = const.tile([2, 2], bf16, tag="id2b")
        nc.vector.tensor_copy(id2b[:], id2[:])

        # ---- collective warm-up ----
        wu_sb = const.tile([P, 128], f32, tag="wu_sb")
        nc.vector.memset(wu_sb[:], 0.0)
        wu_in = dram.tile([P, 128], f32, tag="wu_in", name="wu_in")
        wu_out = dram.tile([P * c.NC, 128], f32, addr_space=SHARED,
                           tag="wu_out", name="wu_out")
        nc.sync.dma_start(out=wu_in[:], in_=wu_sb[:])
        nc.gpsimd.collective_compute(
            "AllGather", OP.bypass, replica_groups=RG,
            ins=[wu_in[:]], outs=[wu_out[:]])

        # ---- persistent activation state ----
        xn = persist.tile([P, c.KT, c.T], fp8, tag="xn")
        s12_col = const.tile([P, 1], f32, tag="s12_col")
        nc.vector.memset(s12_col[:], 1.0 / (W8SCALE * W8SCALE))
        s64_col = const.tile([P, 1], f32, tag="s64_col")
        nc.vector.memset(s64_col[:], 1.0 / W8SCALE)
        xrows = persist.tile([P, c.OT, c.T], f32, tag="xrows")
        for ot in range(c.OT):
            nc.scalar.dma_start(out=xrows[:, ot, :],
                                in_=x0r[ot * P:(ot + 1) * P, :])

        # ---------- helpers ----------
        def emit_norm(src_ap, lnidx, dst, ncols, chunks):
            """rmsnorm of src [H, ncols] (bf16) -> dst [P, KT, ncols]."""
            ss = [psum.tile([1, cw], f32, tag="ps", name=f"ssps{lnidx}_{ci}")
                  for ci, (c0, cw) in enumerate(chunks)]
            for kt in range(c.KT):
                xf = xpool.tile([P, ncols], bf16, tag="xf", name="xf", bufs=2)
                nc.sync.dma_start(out=xf[:], in_=src_ap[kt * P:(kt + 1) * P, :])
                nc.vector.tensor_copy(dst[:, kt, :], xf[:])
                sq = xpool.tile([P, ncols], bf16, tag="sq", name="sq", bufs=2)
                nc.vector.tensor_mul(sq[:], xf[:], xf[:])
                for ci, (c0, cw) in enumerate(chunks):
                    nc.tensor.matmul(ss[ci][:], ones_cbf[:], sq[:, c0:c0 + cw],
                                     start=(kt == 0), stop=(kt == c.KT - 1))
            bc = spool.tile([P, ncols], f32, tag="bc", name="bc", bufs=1)
            for ci, (c0, cw) in enumerate(chunks):
                lt = spool.tile([1, cw], f32, tag="lt", name="lt")
                nc.scalar.activation(lt[:], ss[ci][:], AF.Ln,
                                     bias=eps_col[0:1, :], scale=1.0 / c.H)
                rt = spool.tile([1, cw], f32, tag="rt", name="rt")
                nc.scalar.activation(rt[:], lt[:], AF.Exp, scale=-0.5)
                bb = psum.tile([P, cw], f32, tag="ps", name="bbps")
                nc.tensor.matmul(bb[:], ones_r32[:], rt[:],
                                 start=True, stop=True)
                nc.scalar.copy(bc[:, c0:c0 + cw], bb[:])
            for kt in range(c.KT):
                nc.vector.scalar_tensor_tensor(
                    dst[:, kt, :], dst[:, kt, :],
                    lnw_sb[:, lnidx, kt:kt + 1], bc[:],
                    OP.mult, OP.mult)

        def emit_rstd_bcast(ss_aps, ncols, chunks, pcount=None):
            """per-chunk [1, cw] full-H sumsq APs -> [P, ncols] rstd bcast."""
            pc = P if pcount is None else pcount
            bc = spool.tile([pc, ncols], f32, tag="bc", name="bc", bufs=1)
            for ci, (c0, cw) in enumerate(chunks):
                lt = spool.tile([1, cw], f32, tag="lt", name="lt")
                nc.scalar.activation(lt[:], ss_aps[ci], AF.Ln,
                                     bias=eps_col[0:1, :], scale=1.0 / c.H)
                rt = spool.tile([1, cw], f32, tag="rt", name="rt")
                nc.scalar.activation(rt[:], lt[:], AF.Exp, scale=-0.5)
                bb = psum.tile([pc, cw], f32, tag="ps", name="bbps")
                nc.tensor.matmul(bb[:], ones_r32[:, 0:pc], rt[:],
                                 start=True, stop=True)
                nc.scalar.copy(bc[:, c0:c0 + cw], bb[:])
            return bc

        def emit_sumsq_ar(cols0, ncols, tag):
            """Partial sum-of-squares of this core's fp32 x rows over
            [cols0, cols0+ncols), then a tiny AllReduce."""
            ssq = psum.tile([1, ncols], f32, tag="ps", name=f"ssA{tag}")
            for ot in range(c.OT):
                sqr = xpool.tile([P, ncols], bf16, tag="sq", name="sqr",
                                 bufs=2)
                nc.vector.tensor_mul(sqr[:], xrows[:, ot, cols0:cols0 + ncols],
                                     xrows[:, ot, cols0:cols0 + ncols])
                nc.tensor.matmul(ssq[:], ones_cbf[:], sqr[:],
                                 start=(ot == 0), stop=(ot == c.OT - 1))
            srow = spool.tile([1, ncols], f32, tag="srow", name="srow")
            nc.scalar.copy(srow[:], ssq[:])
            ssb = dram.tile([1, ncols], f32, tag=f"ssb{tag}",
                            name=f"ssb{tag}")
            ssg = dram.tile([1, ncols], f32, addr_space=SHARED,
                            tag=f"ssg{tag}", name=f"ssg{tag}")
            nc.sync.dma_start(out=ssb[:], in_=srow[:])
            nc.gpsimd.collective_compute(
                "AllReduce", OP.add, replica_groups=RG,
                ins=[ssb[:]], outs=[ssg[:]])
            return ssg

        def emit_norm_post(ssg, src_ap, lnidx, dst, cols0, ncols):
            """Normalize src [H, ncols] (bf16, DRAM) into
            dst[:, kt, cols0:cols0+ncols] using the AllReduced sumsq."""
            sst = spool.tile([1, ncols], f32, tag="sst", name="sst")
            nc.sync.dma_start(out=sst[:], in_=ssg[:])
            bc = emit_rstd_bcast([sst[:]], ncols, [(0, ncols)])
            for kt in range(c.KT):
                xf = xpool.tile([P, ncols], bf16, tag="xf", name="xfa",
                                bufs=2)
                nc.sync.dma_start(out=xf[:],
                                  in_=src_ap[kt * P:(kt + 1) * P, :])
                nc.vector.scalar_tensor_tensor(
                    dst[:, kt, cols0:cols0 + ncols], xf[:],
                    lnw_sb[:, lnidx, kt:kt + 1],
                    bc[:], OP.mult, OP.mult)

        def emit_norm_raw_cols(src_ap, dst, cols0, ncols):
            """Copy RAW src [H, ncols] (bf16 DRAM) into dst cols and
            return the rstd broadcast tile (norm folded downstream)."""
            ss = psum.tile([1, ncols], f32, tag="ps", name="ssc")
            for kt in range(c.KT):
                xf = xpool.tile([P, ncols], bf16, tag="xf", name="xfc",
                                bufs=2)
                nc.sync.dma_start(out=xf[:],
                                  in_=src_ap[kt * P:(kt + 1) * P, :])
                nc.vector.tensor_copy(dst[:, kt, cols0:cols0 + ncols],
                                      xf[:])
                sq = xpool.tile([P, ncols], bf16, tag="sq", name="sqc",
                                bufs=2)
                nc.vector.tensor_mul(sq[:], xf[:], xf[:])
                nc.tensor.matmul(ss[:], ones_cbf[:], sq[:],
                                 start=(kt == 0), stop=(kt == c.KT - 1))
            return emit_rstd_bcast([ss[:]], ncols, [(0, ncols)])

        def emit_norm_slim(src_ap, lnidx, dst3):
            """rmsnorm of a [H, B] tensor: one DMA + local sumsq."""
            xs = spool.tile([P, c.KT, c.B], bf16, tag="xs_slim",
                            name="xs_slim")
            nc.sync.dma_start(
                out=xs[:],
                in_=src_ap.rearrange("(kt p) b -> p kt b", p=P))
            sq = spool.tile([P, c.KT, c.B], f32, tag="sq_slim",
                            name="sq_slim")
            nc.vector.tensor_mul(sq[:], xs[:], xs[:])
            sp_ = psum.tile([1, c.KT * c.B], f32, tag="ps", name="spslim")
            nc.tensor.matmul(sp_[:], ones_c32[:],
                             sq[:].rearrange("p kt b -> p (kt b)"),
                             start=True, stop=True)
            ss2 = spool.tile([1, c.B], f32, tag="ss2", name="ss2")
            nc.vector.tensor_reduce(
                ss2[:], sp_[:].rearrange("o (kt b) -> o b kt", b=c.B),
                mybir.AxisListType.X, OP.add)
            bc = emit_rstd_bcast([ss2[:]], c.B, [(0, c.B)])
            tmp = spool.tile([P, c.KT, c.B], f32, tag="tmp_slim",
                             name="tmp_slim")
            nc.vector.tensor_tensor(
                tmp[:], xs[:],
                lnw_sb[:, lnidx, :].unsqueeze(2).broadcast_to(
                    (P, c.KT, c.B)), OP.mult)
            nc.vector.tensor_tensor(
                dst3[:], tmp[:],
                bc[:].unsqueeze(1).broadcast_to((P, c.KT, c.B)), OP.mult)

        def kouter_pass(KK, wsrc, wcols, groups, rhs_fn, rhs_load=None,
                        name="kp"):
            """Generic contraction pass: loop k tiles (batched weight DMA),
            stream weights, accumulate len(groups) psum tiles."""
            ps = [psum.tile([cw, n], f32, tag="ps", name=f"{name}{gi}")
                  for gi, (c0, cw, n, rk) in enumerate(groups)]
            G = max(1, min(4, 2048 // wcols))
            for k0 in range(0, KK, G):
                g_n = min(G, KK - k0)
                wt = wpool.tile([P, G, 2048 // G if wcols > 2048 // G else wcols],
                                bf16, tag="wt", name=f"{name}w")
                weng = nc.sync if (k0 // G) % 2 == 0 else nc.scalar
                weng.dma_start(
                    out=wt[:, 0:g_n, 0:wcols],
                    in_=wsrc(k0, g_n).rearrange("g p m -> p g m"))
                for g in range(g_n):
                    kt = k0 + g
                    rl = rhs_load(kt) if rhs_load is not None else None
                    for gi, (c0, cw, n, rk) in enumerate(groups):
                        nc.tensor.matmul(ps[gi][:], wt[:, g, c0:c0 + cw],
                                         rhs_fn(kt, rk, rl),
                                         start=(kt == 0), stop=(kt == KK - 1))
            return ps

        def qk_pair_pass(wsrc, groups, name):
            """DoubleRow q/k projection: out [cw, S] per group from
            fp8 pair weights and fp8 xn pairs."""
            ps = [psum.tile([cw, n], f32, tag="ps", name=f"{name}{gi}")
                  for gi, (c0, cw, n, rk) in enumerate(groups)]
            for ktp in range(c.KT // 2):
                wt8 = wpool.tile([P, 2, c.DR], fp8, tag="w8",
                                 name=f"{name}w", bufs=3)
                weng = nc.sync if ktp % 2 == 0 else nc.scalar
                weng.dma_start(out=wt8[:], in_=wsrc[ktp])
                for gi, (c0, cw, n, rk) in enumerate(groups):
                    nc.tensor.matmul(
                        ps[gi][:], wt8[:, :, c0:c0 + cw],
                        xn[:, 2 * ktp:2 * ktp + 2,
                           rk * c.S:(rk + 1) * c.S],
                        start=(ktp == 0), stop=(ktp == c.KT // 2 - 1),
                        perf_mode=mybir.MatmulPerfMode.DoubleRow)
            return ps

        def emit_rope(src_ps, qr_dst, cos_ap, sin_ap, ncols):
            """rope: qr_dst = src*cos + swap_half(src)*sin_signed."""
            h2 = c.HD // 2
            qs = rpool.tile([c.HD, ncols], f32, tag="qs", name="qs")
            nc.vector.tensor_copy(qs[:], src_ps[:])
            rot = rpool.tile([c.HD, ncols], f32, tag="rot", name="rot")
            nc.sync.dma_start(out=rot[0:h2, :], in_=qs[h2:c.HD, :])
            nc.sync.dma_start(out=rot[h2:c.HD, :], in_=qs[0:h2, :])
            nc.vector.tensor_mul(qs[:], qs[:], cos_ap)
            nc.vector.tensor_mul(rot[:], rot[:], sin_ap)
            nc.vector.tensor_add(qr_dst, qs[:], rot[:])

        # =====================================================
        # LAYER 0 (full tokens, per-batch pipelined)
        # =====================================================
        l = 0
        full_chunks = [(b * c.S, c.S) for b in range(c.B)]
        # ln1 folded: copy RAW x into xn, rstd applied via rope tables
        # and v-copy scale (ln1_w folded into wq/wk/wv host-side)
        ss1 = [psum.tile([1, cw], f32, tag="ps", name=f"ss1_{ci}")
               for ci, (c0, cw) in enumerate(full_chunks)]
        for kt in range(c.KT):
            xf = xpool.tile([P, c.T], bf16, tag="xf", name="xf", bufs=2)
            nc.sync.dma_start(out=xf[:], in_=x0[kt * P:(kt + 1) * P, :])
            nc.vector.tensor_copy(xn[:, kt, :], xf[:])
            sq = xpool.tile([P, c.T], bf16, tag="sq", name="sq", bufs=2)
            nc.vector.tensor_mul(sq[:], xf[:], xf[:])
            for ci, (c0, cw) in enumerate(full_chunks):
                nc.tensor.matmul(ss1[ci][:], ones_cbf[:], sq[:, c0:c0 + cw],
                                 start=(kt == 0), stop=(kt == c.KT - 1))
        bc1 = emit_rstd_bcast([s[:] for s in ss1], c.T, full_chunks)
        cosb = mpool.tile([c.HD, c.T], bf16, tag="cosb", name="cosb",
                          bufs=1)
        nc.vector.scalar_tensor_tensor(cosb[:], cos_sb[:],
                                       s64_col[0:c.HD, :],
                                       bc1[0:c.HD, :], OP.mult, OP.mult)
        sinb = mpool.tile([c.HD, c.T], bf16, tag="sinb", name="sinb",
                          bufs=1)
        nc.vector.scalar_tensor_tensor(sinb[:], sin_sb[:],
                                       s64_col[0:c.HD, :],
                                       bc1[0:c.HD, :], OP.mult, OP.mult)
        q_rot = persist.tile([c.HD, c.HPC, c.T], bf16, tag="qrot",
                             name=f"qrot{l}")
        k_rot = persist.tile([c.HD, c.HPC, c.T], bf16, tag="krot",
                             name=f"krot{l}")
        v_sb = persist.tile([P, c.TP_, c.HPC, c.HD + 1], bf16, tag="vsb",
                            name=f"vsb{l}")
        nc.vector.memset(v_sb[:, :, :, c.HD:c.HD + 1], 1.0)

        # q pass
        qg = [(h * c.HD, c.HD, c.S, b)
              for h in range(c.HPC) for b in range(c.B)]
        qrhs = lambda kt, rk, rl: xn[:, kt, rk * c.S:(rk + 1) * c.S]
        qps = qk_pair_pass(wq[l], qg, "qp")
        for gi, (c0, cw, n, rk) in enumerate(qg):
            h = c0 // c.HD
            emit_rope(qps[gi], q_rot[:, h, rk * c.S:(rk + 1) * c.S],
                      cosb[:, rk * c.S:(rk + 1) * c.S],
                      sinb[:, rk * c.S:(rk + 1) * c.S], c.S)

        # k pass
        kps = qk_pair_pass(wk[l], qg, "kp")
        # rstd as a token-partition column for the v-copy scale (issued
        # after q/k so the tiny transposes never stall the PE queue)
        rtp = psum.tile([P, c.TP_], f32, tag="ps", name="rtp")
        for tt in range(c.TP_):
            nc.tensor.matmul(rtp[:, tt:tt + 1],
                             bc1[0:1, tt * P:(tt + 1) * P],
                             id2[0:1, 0:1], start=True, stop=True,
                             is_transpose=True)
        rtc = spool.tile([P, c.TP_], f32, tag="rtc", name="rtc")
        nc.scalar.activation(rtc[:], rtp[:], AF.Copy,
                             scale=1.0 / W8SCALE)
        for gi, (c0, cw, n, rk) in enumerate(qg):
            h = c0 // c.HD
            emit_rope(kps[gi], k_rot[:, h, rk * c.S:(rk + 1) * c.S],
                      cosb[:, rk * c.S:(rk + 1) * c.S],
                      sinb[:, rk * c.S:(rk + 1) * c.S], c.S)

        # v pass (token-major)
        vps = [psum.tile([P, c.DR], f32, tag="ps", name=f"vp{tt}")
               for tt in range(c.TP_)]
        DRm = mybir.MatmulPerfMode.DoubleRow
        for ktp in range(c.KT // 2):
            wt = wpool.tile([P, 2, c.DR], fp8, tag="w8", name="vw")
            weng = nc.sync if ktp % 2 == 0 else nc.scalar
            weng.dma_start(out=wt[:], in_=wv[l][ktp])
            for tt in range(c.TP_):
                nc.tensor.matmul(vps[tt][:],
                                 xn[:, 2 * ktp:2 * ktp + 2,
                                    tt * P:(tt + 1) * P],
                                 wt[:],
                                 start=(ktp == 0),
                                 stop=(ktp == c.KT // 2 - 1),
                                 perf_mode=DRm)
        for tt in range(c.TP_):
            for h in range(c.HPC):
                nc.scalar.activation(v_sb[:, tt, h, 0:c.HD],
                                     vps[tt][:, h * c.HD:(h + 1) * c.HD],
                                     AF.Copy, scale=rtc[:, tt:tt + 1])

        # ---- attention (per batch) + ctx AllGather ----
        ctxbs = [dram.tile([c.DR, c.S], fp8, tag=f"ctxb{l}_{b}",
                           name=f"ctxb{l}_{b}") for b in range(c.B)]
        ctxgs = [dram.tile([c.H, c.S], fp8, addr_space=SHARED,
                           tag=f"ctxg{l}_{b}", name=f"ctxg{l}_{b}")
                 for b in range(c.B)]
        def attn_tail(b, h, cps):
            dr = spool.tile([1, c.S], bf16, tag="dr", name="dr")
            with nc.allow_low_precision(reason="attn 1/den bcast"):
                nc.vector.reciprocal(dr[:], cps[c.HD:c.HD + 1, :])
            bb = psum.tile([c.HD, c.S], f32, tag="ps", name="bb")
            nc.tensor.matmul(bb[:], ones_rb[:, 0:c.HD], dr[:],
                             start=True, stop=True)
            bsb = spool.tile([c.HD, c.S], bf16, tag="bsb",
                             name="bsb", bufs=2)
            nc.vector.tensor_copy(bsb[:], bb[:])
            csb = spool.tile([c.HD, c.S], fp8, tag="csb",
                             name="csb", bufs=2)
            nc.vector.tensor_mul(csb[:], cps[0:c.HD, :], bsb[:])
            nc.sync.dma_start(
                out=ctxbs[b][h * c.HD:(h + 1) * c.HD, :],
                in_=csb[:])

        pend = []
        for b in range(c.B):
            for h0 in range(0, c.HPC, 2):
                hs = [h for h in (h0, h0 + 1) if h < c.HPC]
                cpss = {h: psum.tile([c.HD + 1, c.S], f32, tag="ps",
                                     name=f"cps{h - h0}") for h in hs}
                for t in range(c.SP):
                    pts = {}
                    for h in hs:
                        sps = psum.tile([P, c.S], f32, tag="ps",
                                        name=f"sps{h - h0}")
                        nc.tensor.matmul(
                            sps[:],
                            k_rot[:, h,
                                  b * c.S + t * P:b * c.S + (t + 1) * P],
                            q_rot[:, h, b * c.S:(b + 1) * c.S],
                            start=True, stop=True)
                        pt = ppool.tile([P, c.S], bf16, tag="pt",
                                        name=f"pt{h - h0}")
                        nc.scalar.activation(pt[:], sps[:], AF.Exp,
                                             scale=isqrt_hd)
                        m0 = (c.SP - 1 - t) * P
                        nc.vector.scalar_tensor_tensor(
                            pt[:], pt[:], am2_sb[:, b, t:t + 1],
                            mask_sb[:, m0:m0 + c.S], OP.mult, OP.mult)
                        pts[h] = pt
                    for h in hs:
                        nc.tensor.matmul(
                            cpss[h][:],
                            v_sb[:, b * c.SP + t, h, :],
                            pts[h][:],
                            start=(t == 0), stop=(t == c.SP - 1))
                for bp, hp, cp in pend:
                    attn_tail(bp, hp, cp)
                pend = [(b, h, cpss[h]) for h in hs]
            for bp, hp, cp in pend:
                attn_tail(bp, hp, cp)
            pend = []
            nc.gpsimd.collective_compute(
                "AllGather", OP.bypass, replica_groups=RG,
                ins=[ctxbs[b][:]], outs=[ctxgs[b][:]])

        # ---- o projection per batch + boundary collectives ----
        ln2_ssg = [None] * c.B
        xgo_b = [None] * c.B
        for b in range(c.B):
            ops_b = [psum.tile([P, c.S], f32, tag="ps", name=f"op{b}{ot}")
                     for ot in range(c.OT)]
            for ktp in range(c.KT // 2):
                wt8 = wpool.tile([P, 2, c.OR], fp8, tag="w8",
                                 name="ow8", bufs=3)
                weng = nc.sync if ktp % 2 == 0 else nc.scalar
                weng.dma_start(out=wt8[:], in_=wo[l][ktp])
                ct = xpool.tile([P, 2, c.S], fp8, tag="orhs",
                                name="orhs", bufs=2)
                oeng = nc.scalar if ktp % 2 == 0 else nc.sync
                oeng.dma_start(
                    out=ct[:],
                    in_=ctxgs[b][2 * ktp * P:(2 * ktp + 2) * P, :]
                    .rearrange("(k p) s -> p k s", p=P))
                for ot in range(c.OT):
                    nc.tensor.matmul(
                        ops_b[ot][:], wt8[:, :, ot * P:(ot + 1) * P],
                        ct[:], start=(ktp == 0),
                        stop=(ktp == c.KT // 2 - 1),
                        perf_mode=mybir.MatmulPerfMode.DoubleRow)
            xbo = dram.tile([c.OR, c.S], bf16, tag=f"xbo{l}_{b}",
                            name=f"xbo{l}_{b}")
            xgo = dram.tile([c.H, c.S], bf16, addr_space=SHARED,
                            tag=f"xgo{l}_{b}", name=f"xgo{l}_{b}")
            for ot in range(c.OT):
                xsl = xrows[:, ot, b * c.S:(b + 1) * c.S]
                nc.vector.scalar_tensor_tensor(
                    xsl, ops_b[ot][:], s64_col[:], xsl,
                    OP.mult, OP.add)
                st = xpool.tile([P, c.S], bf16, tag="xst", name="xst",
                                bufs=2)
                nc.scalar.copy(st[:], xsl)
                nc.sync.dma_start(out=xbo[ot * P:(ot + 1) * P, :], in_=st[:])
            ln2_ssg[b] = emit_sumsq_ar(b * c.S, c.S, tag=f"o{l}{b}")
            nc.gpsimd.collective_compute(
                "AllGather", OP.bypass, replica_groups=RG,
                ins=[xbo[:]], outs=[xgo[:]])
            xgo_b[b] = xgo

        # ---- MLP per batch: ln2, gate/up (local), down (input-sharded),
        #      ReduceScatter, residual, sumsq AR, x AllGather ----
        ln1_ssg = [None] * c.B
        xgd_b = [None] * c.B
        rsouts = [None] * c.B
        rsins = [None] * c.B

        it_sbs = [None] * c.B

        def mlp_gu(b):
            emit_norm_post(ln2_ssg[b], xgo_b[b][:], 2 * l + 1, xn,
                           b * c.S, c.S)
            gact = mpool.tile([P, c.FT, c.S], bf16, tag="gact",
                              name=f"gact{l}_{b}", bufs=1)
            it_sb = mpool.tile([P, c.FT, c.S], fp8, tag="it_sb",
                               name=f"it{l}_{b}", bufs=1)
            it_sbs[b] = it_sb
            DR_ = mybir.MatmulPerfMode.DoubleRow
            for phase, wsrc3 in (("g", wg[l]), ("u", wu[l])):
                gps = [psum.tile([P, c.S], f32, tag="ps",
                                 name=f"{phase}{b}_{ot}")
                       for ot in range(c.FT)]
                for ktp in range(c.KT // 2):
                    wt8 = wpool.tile([P, 2, c.FPC], fp8, tag="w8",
                                     name=f"{phase}w8", bufs=3)
                    weng = nc.sync if ktp % 2 == 0 else nc.scalar
                    weng.dma_start(out=wt8[:], in_=wsrc3[ktp])
                    for ot in range(c.FT):
                        nc.tensor.matmul(
                            gps[ot][:],
                            wt8[:, :, ot * P:(ot + 1) * P],
                            xn[:, 2 * ktp:2 * ktp + 2,
                               b * c.S:(b + 1) * c.S],
                            start=(ktp == 0), stop=(ktp == c.KT // 2 - 1),
                            perf_mode=DR_)
                for ot in range(c.FT):
                    if phase == "g":
                        sgt = xpool.tile([P, c.S], bf16, tag="sgt",
                                         name="sgt", bufs=2)
                        nc.scalar.activation(sgt[:], gps[ot][:], AF.Sigmoid,
                                             scale=1.0 / W8SCALE)
                        nc.vector.tensor_mul(
                            gact[:, ot, :], gps[ot][:], sgt[:])
                    else:
                        nc.vector.scalar_tensor_tensor(
                            it_sb[:, ot, :], gps[ot][:], s12_col[:],
                            gact[:, ot, :], OP.mult, OP.mult)

        def mlp_down(b):
            it_sb = it_sbs[b]
            # down: input-sharded over this core's FPC block, out = full H
            rsin = dram.tile([c.H, c.S], bf16, tag=f"rsin{l}_{b}",
                             name=f"rsin{l}_{b}")
            rsout = dram.tile([c.OR, c.S], bf16, tag=f"rsout{l}_{b}",
                              name=f"rsout{l}_{b}")
            DR_ = mybir.MatmulPerfMode.DoubleRow
            for ot in range(c.KT):
                wsb = wpool.tile([P, c.FT // 2, 2, P], fp8, tag="w8",
                                 name="wdw", bufs=3)
                weng = nc.sync if ot % 2 == 0 else nc.scalar
                weng.dma_start(out=wsb[:], in_=wd[l][ot])
                dps = psum.tile([P, c.S], f32, tag="ps", name="dps")
                for ktp in range(c.FT // 2):
                    nc.tensor.matmul(
                        dps[:], wsb[:, ktp, :, :],
                        it_sb[:, 2 * ktp:2 * ktp + 2, :],
                        start=(ktp == 0), stop=(ktp == c.FT // 2 - 1),
                        perf_mode=DR_)
                dp = xpool.tile([P, c.S], bf16, tag="dp", name="dp", bufs=2)
                if ot % 2 == 0:
                    nc.scalar.activation(dp[:], dps[:], AF.Copy,
                                         scale=1.0 / W8SCALE)
                else:
                    nc.vector.tensor_scalar_mul(dp[:], dps[:],
                                                1.0 / W8SCALE)
                nc.sync.dma_start(out=rsin[ot * P:(ot + 1) * P, :],
                                  in_=dp[:])
            rsins[b] = rsin
            rsouts[b] = rsout

        def mlp_rs(b):
            nc.gpsimd.collective_compute(
                "ReduceScatter", OP.add, replica_groups=RG,
                ins=[rsins[b][:]], outs=[rsouts[b][:]])

        def mlp_tail(b):
            """Residual add of the RS output, then x AG (no PE work)."""
            rso = xpool.tile([P, c.OT, c.S], bf16, tag="rso", name="rso",
                             bufs=1)
            for ot in range(c.OT):
                nc.sync.dma_start(out=rso[:, ot, :],
                                  in_=rsouts[b][ot * P:(ot + 1) * P, :])
            xbd = dram.tile([c.OR, c.S], bf16, tag=f"xbd{l}_{b}",
                            name=f"xbd{l}_{b}")
            xgd = dram.tile([c.H, c.S], bf16, addr_space=SHARED,
                            tag=f"xgd{l}_{b}", name=f"xgd{l}_{b}")
            for ot in range(c.OT):
                xsl = xrows[:, ot, b * c.S:(b + 1) * c.S]
                nc.vector.tensor_add(xsl, xsl, rso[:, ot, :])
                st = xpool.tile([P, c.S], bf16, tag="xst", name="xst2",
                                bufs=2)
                nc.scalar.copy(st[:], xsl)
                nc.sync.dma_start(out=xbd[ot * P:(ot + 1) * P, :], in_=st[:])
            nc.gpsimd.collective_compute(
                "AllGather", OP.bypass, replica_groups=RG,
                ins=[xbd[:]], outs=[xgd[:]])
            xgd_b[b] = xgd

        # =====================================================
        # LAYER 1 (slim: k/v full per batch; q/attn/o/MLP last-token)
        # =====================================================
        q_rot2 = persist.tile([c.HD, c.HPC, c.B], bf16, tag="qrot2",
                              name="qrot2")
        k_rot2 = persist.tile([c.HD, c.HPC, c.T], bf16, tag="krot",
                              name="krot2")
        v_sb2 = persist.tile([P, c.TP_, c.HPC, c.HD + 1], bf16, tag="vsb",
                             name="vsb2")
        nc.vector.memset(v_sb2[:, :, :, c.HD:c.HD + 1], 1.0)

        rlastc = persist.tile([c.HD, c.B], f32, tag="rlastc",
                              name="rlastc")
        rlasts = persist.tile([c.HD, c.B], f32, tag="rlasts",
                              name="rlasts")

        def l1_kv(b):
            l = LAST
            bcb = emit_norm_raw_cols(xgd_b[b][:], xn, b * c.S, c.S)
            # rstd-scaled rope tables for this batch + last-token tables
            nc.vector.scalar_tensor_tensor(
                cosb[:, b * c.S:(b + 1) * c.S],
                cos_sb[:, b * c.S:(b + 1) * c.S],
                s64_col[0:c.HD, :], bcb[0:c.HD, :], OP.mult, OP.mult)
            nc.vector.scalar_tensor_tensor(
                sinb[:, b * c.S:(b + 1) * c.S],
                sin_sb[:, b * c.S:(b + 1) * c.S],
                s64_col[0:c.HD, :], bcb[0:c.HD, :], OP.mult, OP.mult)
            nc.vector.scalar_tensor_tensor(
                rlastc[:, b:b + 1], cos2_sb[:, b:b + 1],
                s64_col[0:c.HD, :], bcb[0:c.HD, c.S - 1:c.S],
                OP.mult, OP.mult)
            nc.vector.scalar_tensor_tensor(
                rlasts[:, b:b + 1], sin2_sb[:, b:b + 1],
                s64_col[0:c.HD, :], bcb[0:c.HD, c.S - 1:c.S],
                OP.mult, OP.mult)
            # k pass for this batch
            kg_b = [(h * c.HD, c.HD, c.S, b) for h in range(c.HPC)]
            krhs = (lambda kt, rk, rl, _b=b:
                    xn[:, kt, _b * c.S:(_b + 1) * c.S])
            kps = qk_pair_pass(wk[l], kg_b, f"kp2{b}")
            # rstd column for the v-copy scale (after k: no PE stall)
            rtp = psum.tile([P, c.SP], f32, tag="ps", name=f"rtp2{b}")
            for tt in range(c.SP):
                nc.tensor.matmul(
                    rtp[:, tt:tt + 1],
                    bcb[0:1, tt * P:(tt + 1) * P],
                    id2[0:1, 0:1], start=True, stop=True,
                    is_transpose=True)
            rtc2 = spool.tile([P, c.SP], f32, tag="rtc2", name="rtc2",
                              bufs=2)
            nc.scalar.activation(rtc2[:], rtp[:], AF.Copy,
                                 scale=1.0 / W8SCALE)
            for gi, (c0, cw, n, rk) in enumerate(kg_b):
                h = c0 // c.HD
                emit_rope(kps[gi], k_rot2[:, h, b * c.S:(b + 1) * c.S],
                          cosb[:, b * c.S:(b + 1) * c.S],
                          sinb[:, b * c.S:(b + 1) * c.S], c.S)
            # v pass for this batch
            vps = [psum.tile([P, c.DR], f32, tag="ps", name=f"vp2{b}{tt}")
                   for tt in range(c.SP)]
            DRm = mybir.MatmulPerfMode.DoubleRow
            for ktp in range(c.KT // 2):
                wt = wpool.tile([P, 2, c.DR], fp8, tag="w8", name="vw2")
                weng = nc.sync if ktp % 2 == 0 else nc.scalar
                weng.dma_start(out=wt[:], in_=wv[l][ktp])
                for tt in range(c.SP):
                    gt = b * c.SP + tt
                    nc.tensor.matmul(vps[tt][:],
                                     xn[:, 2 * ktp:2 * ktp + 2,
                                        gt * P:(gt + 1) * P],
                                     wt[:],
                                     start=(ktp == 0),
                                     stop=(ktp == c.KT // 2 - 1),
                                     perf_mode=DRm)
            for tt in range(c.SP):
                for h in range(c.HPC):
                    nc.scalar.activation(
                        v_sb2[:, b * c.SP + tt, h, 0:c.HD],
                        vps[tt][:, h * c.HD:(h + 1) * c.HD],
                        AF.Copy, scale=rtc2[:, tt:tt + 1])

        # interleave: tail(0) hides under gu(1)/down(1); RS(1)+tail(1)
        # hide under layer-1 b0 k/v
        mlp_gu(0)
        mlp_down(0)
        mlp_rs(0)
        mlp_gu(1)
        mlp_down(1)
        mlp_tail(0)
        mlp_rs(1)
        l1_kv(0)
        mlp_tail(1)
        l1_kv(1)
        l = LAST

        # q pass (last tokens only)
        qg2 = [(h * c.HD, c.HD, c.B, 0) for h in range(c.HPC)]
        qps2 = [psum.tile([c.HD, c.B], f32, tag="ps", name=f"qp2{gi}")
                for gi in range(c.HPC)]
        for ktp in range(c.KT // 2):
            wt8 = wpool.tile([P, 2, c.DR], fp8, tag="w8", name="qp2w",
                             bufs=3)
            weng = nc.sync if ktp % 2 == 0 else nc.scalar
            weng.dma_start(out=wt8[:], in_=wq[l][ktp])
            rhs2 = xn[:, 2 * ktp:2 * ktp + 2, :].rearrange(
                "p k (b s) -> p k b s", s=c.S)[:, :, :, c.S - 1]
            for gi, (c0, cw, n, rk) in enumerate(qg2):
                nc.tensor.matmul(
                    qps2[gi][:], wt8[:, :, c0:c0 + cw], rhs2,
                    start=(ktp == 0), stop=(ktp == c.KT // 2 - 1),
                    perf_mode=mybir.MatmulPerfMode.DoubleRow)
        for gi, (c0, cw, n, rk) in enumerate(qg2):
            h = c0 // c.HD
            emit_rope(qps2[gi], q_rot2[:, h, :], rlastc[:], rlasts[:], c.B)

        # ---- replicated last-token state x2 [P, KT, B] fp32 ----
        OTO = c.DR // P
        x2 = persist.tile([P, c.KT, c.B], f32, tag="x2", name="x2")
        x2st = spool.tile([P, c.KT, c.B], bf16, tag="x2st", name="x2st")
        for b in range(c.B):
            nc.sync.dma_start(
                out=x2st[:, :, b],
                in_=xgd_b[b].rearrange("(kt p) s -> p kt s", p=P)
                [:, :, c.S - 1])
        nc.vector.tensor_copy(x2[:], x2st[:])

        def fchunks(total, w=512):
            return [(o, min(w, total - o)) for o in range(0, total, w)]

        def emit_norm_slim_sb(xs, lnidx, dst3):
            """rmsnorm of sbuf fp32 [P, KT, B] -> dst3 bf16."""
            sq = spool.tile([P, c.KT, c.B], f32, tag="sq_slim",
                            name="sq_slim")
            nc.vector.tensor_mul(sq[:], xs[:], xs[:])
            sp_ = psum.tile([1, c.KT * c.B], f32, tag="ps", name="spslim")
            nc.tensor.matmul(sp_[:], ones_c32[:],
                             sq[:].rearrange("p kt b -> p (kt b)"),
                             start=True, stop=True)
            ss2 = spool.tile([1, c.B], f32, tag="ss2", name="ss2")
            nc.vector.tensor_reduce(
                ss2[:], sp_[:].rearrange("o (kt b) -> o b kt", b=c.B),
                mybir.AxisListType.X, OP.add)
            bc = emit_rstd_bcast([ss2[:]], c.B, [(0, c.B)])
            tmp = spool.tile([P, c.KT, c.B], f32, tag="tmp_slim",
                             name="tmp_slim")
            nc.vector.tensor_tensor(
                tmp[:], xs[:],
                lnw_sb[:, lnidx, :].unsqueeze(2).broadcast_to(
                    (P, c.KT, c.B)), OP.mult)
            nc.vector.tensor_tensor(
                dst3[:], tmp[:],
                bc[:].unsqueeze(1).broadcast_to((P, c.KT, c.B)), OP.mult)

        def emit_rstd_col(xs, name):
            """fp32 [P, KT, B] -> [B, 1] rsqrt(mean+eps) column (PE work
            is two tiny ops; scalar chain runs in parallel)."""
            sq = spool.tile([P, c.KT, c.B], f32, tag="sq_slim",
                            name=f"sqr{name}")
            nc.vector.tensor_mul(sq[:], xs[:], xs[:])
            sp_ = psum.tile([1, c.KT * c.B], f32, tag="ps",
                            name=f"sp{name}")
            nc.tensor.matmul(sp_[:], ones_c32[:],
                             sq[:].rearrange("p kt b -> p (kt b)"),
                             start=True, stop=True)
            ss2 = spool.tile([1, c.B], f32, tag="ss2", name=f"ss{name}")
            nc.vector.tensor_reduce(
                ss2[:], sp_[:].rearrange("o (kt b) -> o b kt", b=c.B),
                mybir.AxisListType.X, OP.add)
            lt = spool.tile([1, c.B], f32, tag="lt", name=f"lt{name}")
            nc.scalar.activation(lt[:], ss2[:], AF.Ln,
                                 bias=eps_col[0:1, :], scale=1.0 / c.H)
            rt = spool.tile([1, c.B], f32, tag="rt", name=f"rt{name}")
            nc.scalar.activation(rt[:], lt[:], AF.Exp, scale=-0.5)
            rcp = psum.tile([c.B, 1], f32, tag="ps", name=f"rc{name}")
            nc.tensor.matmul(rcp[:], rt[:], id2[0:1, 0:1],
                             start=True, stop=True, is_transpose=True)
            rc = spool.tile([c.B, 1], f32, tag="rc", name=f"rcc{name}",
                            bufs=2)
            nc.scalar.copy(rc[:], rcp[:])
            return rc

        def emit_slim_ar(src_fn, nchunks_w, arname, wsrc_fn, kts, lhsT_sb,
                         pscale=1.0):
            """Token-major projection out[B, H] = lhsT.T @ W, AllReduce,
            and return a [P, KT, B] stage tile of the result."""
            arin = dram.tile([c.B, c.H], bf16, tag=f"arin{arname}",
                             name=f"arin{arname}")
            arout = dram.tile([c.B, c.H], bf16, addr_space=SHARED,
                              tag=f"arout{arname}", name=f"arout{arname}")
            ocs = fchunks(c.H)
            psl = [psum.tile([c.B, cw], f32, tag="ps",
                             name=f"{arname}ps{oc}")
                   for oc, (c0, cw) in enumerate(ocs)]
            di = 0
            for kt in range(kts):
                for h0, hw in fchunks(c.H, 1536):
                    wt = wpool.tile([P, 1536], fp8, tag="w8",
                                    name=f"{arname}w", bufs=3)
                    eng = nc.sync if di % 2 == 0 else nc.scalar
                    di += 1
                    eng.dma_start(out=wt[:, 0:hw],
                                  in_=wsrc_fn(kt)[:, h0:h0 + hw])
                    for oc, (c0, cw) in enumerate(ocs):
                        if c0 < h0 or c0 >= h0 + hw:
                            continue
                        nc.tensor.matmul(psl[oc][:], lhsT_sb(kt),
                                         wt[:, c0 - h0:c0 - h0 + cw],
                                         start=(kt == 0),
                                         stop=(kt == kts - 1))
            for oc, (c0, cw) in enumerate(ocs):
                osl = spool.tile([c.B, 512], bf16, tag="osl", name="osl",
                                 bufs=2)
                if oc % 2 == 0:
                    nc.scalar.activation(osl[:, 0:cw], psl[oc][:], AF.Copy,
                                         scale=pscale)
                elif isinstance(pscale, float):
                    nc.vector.tensor_scalar_mul(osl[:, 0:cw], psl[oc][:],
                                                pscale)
                else:
                    nc.vector.tensor_scalar(osl[:, 0:cw], psl[oc][:],
                                            pscale, None, OP.mult)
                nc.sync.dma_start(out=arin[:, c0:c0 + cw],
                                  in_=osl[:, 0:cw])
            nc.gpsimd.collective_compute(
                "AllReduce", OP.add, replica_groups=RG,
                ins=[arin[:]], outs=[arout[:]])
            stage = spool.tile([P, c.KT, c.B], bf16, tag="arstage",
                               name=f"arst{arname}", bufs=2)
            for t in range(c.B):
                nc.sync.dma_start(
                    out=stage[:, :, t],
                    in_=arout[t, :].rearrange("(kt p) -> p kt", p=P))
            return stage

        # ---- slim attention: all 8 units interleaved, packed psums ----
        cpk = persist.tile([P, OTO, c.B], bf16, tag="cpk", name="cpk")
        NU = c.B * c.HPC
        spsA = psum.tile([P, NU, c.SP], f32, tag="ps", name="spsA")
        for u in range(NU):
            b, h = u // c.HPC, u % c.HPC
            for t in range(c.SP):
                nc.tensor.matmul(
                    spsA[:, u, t:t + 1],
                    k_rot2[:, h, b * c.S + t * P:b * c.S + (t + 1) * P],
                    q_rot2[:, h, b:b + 1],
                    start=True, stop=True)
        ptA = ppool.tile([P, NU, c.SP], bf16, tag="ptA", name="ptA")
        for u in range(NU):
            b = u // c.HPC
            nc.scalar.activation(ptA[:, u, :], spsA[:, u, :], AF.Exp,
                                 scale=isqrt_hd)
            nc.vector.tensor_mul(ptA[:, u, :], ptA[:, u, :],
                                 am2_sb[:, b, :])
        cpsA = psum.tile([c.HD + 1, NU], f32, tag="ps", name="cpsA")
        for u in range(NU):
            b, h = u // c.HPC, u % c.HPC
            for t in range(c.SP):
                nc.tensor.matmul(
                    cpsA[:, u:u + 1],
                    v_sb2[:, b * c.SP + t, h, :],
                    ptA[:, u, t:t + 1],
                    start=(t == 0), stop=(t == c.SP - 1))
        rA = spool.tile([1, NU], f32, tag="rA", name="rA")
        nc.vector.reciprocal(rA[:], cpsA[c.HD:c.HD + 1, :])
        bbA = psum.tile([c.HD, NU], f32, tag="ps", name="bbA")
        nc.tensor.matmul(bbA[:], ones_r32[:, 0:c.HD], rA[:],
                         start=True, stop=True)
        bsbA = spool.tile([c.HD, NU], f32, tag="bsbA", name="bsbA")
        nc.vector.tensor_copy(bsbA[:], bbA[:])
        csbA = spool.tile([c.HD, NU], bf16, tag="csbA", name="csbA")
        nc.vector.tensor_mul(csbA[:], cpsA[0:c.HD, :], bsbA[:])
        for u in range(NU):
            b, h = u // c.HPC, u % c.HPC
            f0, srcp = h * c.HD, 0
            rem = c.HD
            while rem > 0:
                kt, po = f0 // P, f0 % P
                n = min(P - po, rem)
                nc.sync.dma_start(
                    out=cpk[po:po + n, kt, b:b + 1],
                    in_=csbA[srcp:srcp + n, u:u + 1])
                f0 += n
                srcp += n
                rem -= n

        # ---- slim o projection: token-major partial + AllReduce ----
        ost = emit_slim_ar(None, None, "o",
                           lambda kt: wo2s[kt], OTO,
                           lambda kt: cpk[:, kt, :],
                           pscale=1.0 / W8SCALE)
        nc.vector.tensor_add(x2[:], x2[:], ost[:])

        # ---- slim ln2 (rstd folded into sigmoid/down scales) + MLP ----
        xn2 = persist.tile([P, c.KT, c.B], bf16, tag="xn2", name="xn2")
        nc.vector.tensor_copy(xn2[:], x2[:])
        rc2 = emit_rstd_col(x2, "n2")

        FC = fchunks(c.FPC)
        gps2 = [psum.tile([c.B, cw], f32, tag="ps", name=f"g2_{j}")
                for j, (c0, cw) in enumerate(FC)]
        ups2 = [psum.tile([c.B, cw], f32, tag="ps", name=f"u2_{j}")
                for j, (c0, cw) in enumerate(FC)]
        for kt in range(c.KT):
            wgut = wpool.tile([P, 2 * c.FPC], fp8, tag="w8", name="wguw",
                              bufs=3)
            weng = nc.scalar if (kt < 3 or kt % 2 == 1) else nc.sync
            weng.dma_start(out=wgut[:], in_=wgu1[kt])
            for j, (c0, cw) in enumerate(FC):
                nc.tensor.matmul(gps2[j][:], xn2[:, kt, :],
                                 wgut[:, c0:c0 + cw],
                                 start=(kt == 0), stop=(kt == c.KT - 1))
            for j, (c0, cw) in enumerate(FC):
                nc.tensor.matmul(ups2[j][:], xn2[:, kt, :],
                                 wgut[:, c.FPC + c0:c.FPC + c0 + cw],
                                 start=(kt == 0), stop=(kt == c.KT - 1))
        rsig = spool.tile([c.B, 1], f32, tag="rsig", name="rsig")
        nc.vector.tensor_scalar_mul(rsig[:], rc2[:], 1.0 / W8SCALE)
        rdwn = spool.tile([c.B, 1], f32, tag="rdwn", name="rdwn")
        nc.vector.tensor_mul(rdwn[:], rc2[:], rc2[:])
        nc.vector.tensor_scalar_mul(rdwn[:], rdwn[:], 1.0 / W8SCALE ** 3)
        it2 = spool.tile([c.B, c.FPC], bf16, tag="it2", name="it2")
        for j, (c0, cw) in enumerate(FC):
            sg2 = spool.tile([c.B, 512], bf16, tag="sg2", name="sg2",
                             bufs=2)
            nc.scalar.activation(sg2[:, 0:cw], gps2[j][:], AF.Sigmoid,
                                 scale=rsig[:])
            ga2 = spool.tile([c.B, 512], bf16, tag="ga2", name="ga2",
                             bufs=2)
            nc.vector.tensor_mul(ga2[:, 0:cw], gps2[j][:], sg2[:, 0:cw])
            nc.vector.tensor_mul(it2[:, c0:c0 + cw], ups2[j][:],
                                 ga2[:, 0:cw])
        # transpose int [B, FPC] -> [P, FT, B] via PE
        intp = psum.tile([P, c.FT, c.B], bf16, tag="ps", name="intp")
        for j2 in range(c.FT):
            nc.tensor.matmul(intp[:, j2, :], it2[:, j2 * P:(j2 + 1) * P],
                             id2b[:], start=True, stop=True,
                             is_transpose=True)
        intT = spool.tile([P, c.FT, c.B], bf16, tag="intT", name="intT")
        nc.vector.tensor_copy(intT[:], intp[:])

        # ---- slim down: token-major partial + AllReduce ----
        dst_ = emit_slim_ar(None, None, "d",
                            lambda kt: wd[l][kt], c.FT,
                            lambda kt: intT[:, kt, :],
                            pscale=rdwn[:])
        nc.vector.tensor_add(x2[:], x2[:], dst_[:])

        # ================= final norm (folded) + cls head =================
        xnf = persist.tile([P, c.KT, c.B], bf16, tag="xnf", name="xnf")
        nc.vector.tensor_copy(xnf[:], x2[:])
        rc3 = emit_rstd_col(x2, "nf")

        CC1 = fchunks(c.CLS)
        hps = [psum.tile([c.B, cw], f32, tag="ps", name=f"hps{j}")
               for j, (c0, cw) in enumerate(CC1)]
        for kt in range(c.KT):
            wt = wpool.tile([P, c.CLS], bf16, tag="wt", name="w1w",
                            bufs=3)
            w1eng = nc.scalar if (kt < 3 or kt % 2 == 1) else nc.sync
            w1eng.dma_start(out=wt[:], in_=w1t[kt])
            for j, (c0, cw) in enumerate(CC1):
                nc.tensor.matmul(hps[j][:], xnf[:, kt, :],
                                 wt[:, c0:c0 + cw],
                                 start=(kt == 0), stop=False)
        binv = spool.tile([c.B, 1], f32, tag="binv", name="binv")
        nc.vector.reciprocal(binv[:], rc3[:])
        bivp = psum.tile([1, c.B], f32, tag="ps", name="bivp")
        nc.tensor.matmul(bivp[:], binv[:], id2[:], start=True, stop=True,
                         is_transpose=True)
        binr = spool.tile([1, c.B], bf16, tag="binr", name="binr")
        nc.scalar.copy(binr[:], bivp[:])
        for j, (c0, cw) in enumerate(CC1):
            nc.tensor.matmul(hps[j][:], binr[:],
                             b1_sb[:, c0:c0 + cw],
                             start=False, stop=True)
        h2 = spool.tile([c.B, c.CLS], bf16, tag="h2", name="h2")
        for j, (c0, cw) in enumerate(CC1):
            nc.scalar.activation(h2[:, c0:c0 + cw], hps[j][:], AF.Relu,
                                 scale=rc3[:])
        hq2 = spool.tile([c.B, c.CLS], f32, tag="hq2", name="hq2")
        nc.vector.tensor_mul(hq2[:], h2[:], h2[:])
        mrow = spool.tile([c.B, 1], f32, tag="mrow", name="mrow")
        nc.vector.tensor_reduce(mrow[:], h2[:], mybir.AxisListType.X,
                                OP.add)
        srow2 = spool.tile([c.B, 1], f32, tag="srow2", name="srow2")
        nc.vector.tensor_reduce(srow2[:], hq2[:], mybir.AxisListType.X,
                                OP.add)
        m_sb = spool.tile([c.B, 1], f32, tag="m_sb", name="m_sb")
        nc.vector.tensor_scalar_mul(m_sb[:], mrow[:], 1.0 / c.CLS)
        s_sb = spool.tile([c.B, 1], f32, tag="s_sb", name="s_sb")
        nc.vector.tensor_scalar_mul(s_sb[:], srow2[:], 1.0 / c.CLS)
        msq = spool.tile([c.B, 1], f32, tag="msq", name="msq")
        nc.vector.tensor_mul(msq[:], m_sb[:], m_sb[:])
        var = spool.tile([c.B, 1], f32, tag="var", name="var")
        nc.vector.tensor_sub(var[:], s_sb[:], msq[:])
        lv = spool.tile([c.B, 1], f32, tag="lv", name="lv")
        nc.scalar.activation(lv[:], var[:], AF.Ln, bias=eps_col[0:c.B, :])
        rstd = spool.tile([c.B, 1], f32, tag="rstd", name="rstd")
        nc.scalar.activation(rstd[:], lv[:], AF.Exp, scale=-0.5)
        hn = spool.tile([c.B, c.CLS], bf16, tag="hn", name="hn")
        nc.vector.tensor_scalar(hn[:], h2[:], m_sb[:], rstd[:],
                                OP.subtract, OP.mult)
        # transpose hn [B, CLS] -> [P, CT, B], then logits
        hTp = psum.tile([P, c.CT, c.B], bf16, tag="ps", name="hTp")
        for j2 in range(c.CT):
            nc.tensor.matmul(hTp[:, j2, :], hn[:, j2 * P:(j2 + 1) * P],
                             id2b[:], start=True, stop=True,
                             is_transpose=True)
        hT = spool.tile([P, c.CT, c.B], bf16, tag="hT", name="hT")
        nc.vector.tensor_copy(hT[:], hTp[:])
        w2w = wpool.tile([P, c.CT, c.NCLS], bf16, tag="w2w", name="w2w")
        nc.sync.dma_start(out=w2w[:],
                          in_=w2g[:].rearrange("g p m -> p g m"))
        lg = psum.tile([c.B, c.NCLS], f32, tag="ps", name="lg")
        for j2 in range(c.CT):
            nc.tensor.matmul(lg[:], hT[:, j2, :], w2w[:, j2, :],
                             start=(j2 == 0), stop=False)
        nc.tensor.matmul(lg[:], ones_r2[:, 0:c.B], b2_sb[:],
                         start=False, stop=True)
        lg_sb = spool.tile([c.B, c.NCLS], f32, tag="lg_sb", name="lg_sb")
        nc.vector.tensor_copy(lg_sb[:], lg[:])
        nc.sync.dma_start(out=out_d.rearrange("cc b -> b cc"),
                          in_=lg_sb[:])

    nc.compile()
    return nc


# ----------------------------------------------------------------------------
# entry point
# ----------------------------------------------------------------------------

_CACHE = {}


def _get_nc(cfg):
    if cfg not in _CACHE:
        _CACHE[cfg] = build_nc(cfg)
    return _CACHE[cfg]


def run(cfg, inputs, trace=False, **kw):
    from concourse.bass_utils import run_bass_kernel_spmd
    in_maps = host_prep(cfg, inputs)
    nc = _get_nc(cfg)
    res = run_bass_kernel_spmd(nc, in_maps, core_ids=list(range(cfg.NC)),
                               trace=trace, **kw)
    out = np.asarray(res.results[0]["logits_out"])  # [NCLS, B]
    return np.ascontiguousarray(out.T.astype(np.float32)), res


def kernel(**inputs):
    inputs = {k: np.asarray(v) for k, v in inputs.items()}
    out, _ = run(FULL_CFG, inputs)
    return out
